# revision 1
# baseline (speedup 1.0000x reference)
"""nn_DiTBlock on 8 TRN2 NeuronCores: data-parallel over batch (B=8), one
batch element per core. Self-contained: builds the Bass/Tile kernel, shards
inputs on the host (transpose/pack/cast only), runs SPMD via bass2jax/PJRT,
gathers and un-transposes the output. See build_dit for the on-chip design."""

import numpy as np
from contextlib import ExitStack

import concourse.bass as bass
import concourse.mybir as mybir
import concourse.tile as tile
from concourse import bacc


F32 = mybir.dt.float32
F32R = mybir.dt.float32r
BF16 = mybir.dt.bfloat16
AF = mybir.ActivationFunctionType
OP = mybir.AluOpType

B, T, D, H = 8, 1024, 1024, 16
HD = D // H          # 64
DM = 4 * D           # 4096
NCH = D // 128       # 8
P = 128


def host_prep(x, c, g1, g2, gq, gk, Wqkv, bqkv, Wproj, bproj,
              Wfc1, bfc1, Wfc2, bfc2, Wada, bada):
    import ml_dtypes
    def packT(W, npdt):  # (F, K) -> (K//128, 128, F) contiguous
        Wt = np.ascontiguousarray(np.asarray(W).T).astype(npdt)
        K, F = Wt.shape
        return np.ascontiguousarray(Wt.reshape(K // 128, 128, F))

    f32 = np.float32
    com = {
        "wqkv": packT(Wqkv, f32), "wproj": packT(Wproj, f32),
        "wfc1": packT(Wfc1, f32), "wfc2": packT(Wfc2, f32),
        "wada": packT(Wada, ml_dtypes.bfloat16),
        "bqkv": np.asarray(bqkv, f32), "bproj": np.asarray(bproj, f32),
        "bfc1": np.asarray(bfc1, f32), "bfc2": np.asarray(bfc2, f32),
        "bada": np.asarray(bada, f32),
        "g": np.stack([np.asarray(g1)[0], np.asarray(g2)[0],
                       np.asarray(gq)[0], np.asarray(gk)[0]]).astype(f32),
    }
    in_maps = []
    for b in range(B):
        m = dict(com)
        m["xt"] = np.ascontiguousarray(np.asarray(x[b], f32).T)
        m["cvec"] = np.asarray(c[b], f32)
        in_maps.append(m)
    return in_maps


def host_post(results):
    return np.ascontiguousarray(
        np.stack([r["out"].T for r in results]).astype(np.float32))


def col_ap(handle, nch):
    """DRAM (nch*128,) viewed as [128, nch]: tile[p, ch] = v[ch*128+p]."""
    return bass.AP(tensor=handle, offset=0, ap=[[1, P], [P, nch]])


def bc_ap(handle, n, offset=0):
    """DRAM (n,) broadcast-read to [128, n] (partition stride 0)."""
    return bass.AP(tensor=handle, offset=offset, ap=[[0, P], [1, n]])


def build_dit(n_cores=8, mm_dt=F32R, mlp_dt=F32R, repeat=1):
    nc = bacc.Bacc("TRN2", target_bir_lowering=False, debug=False,
                   num_devices=n_cores)

    xt = nc.dram_tensor("xt", [D, T], F32, kind="ExternalInput")
    cin = nc.dram_tensor("cvec", [D], F32, kind="ExternalInput")
    g = nc.dram_tensor("g", [4], F32, kind="ExternalInput")
    wqkv = nc.dram_tensor("wqkv", [NCH, P, 3 * D], mm_dt, kind="ExternalInput")
    wproj = nc.dram_tensor("wproj", [NCH, P, D], mm_dt, kind="ExternalInput")
    wfc1 = nc.dram_tensor("wfc1", [NCH, P, DM], mlp_dt, kind="ExternalInput")
    wfc2 = nc.dram_tensor("wfc2", [DM // P, P, D], mlp_dt, kind="ExternalInput")
    wada = nc.dram_tensor("wada", [NCH, P, 6 * D], BF16, kind="ExternalInput")
    bqkv = nc.dram_tensor("bqkv", [3 * D], F32, kind="ExternalInput")
    bproj = nc.dram_tensor("bproj", [D], F32, kind="ExternalInput")
    bfc1 = nc.dram_tensor("bfc1", [DM], F32, kind="ExternalInput")
    bfc2 = nc.dram_tensor("bfc2", [D], F32, kind="ExternalInput")
    bada = nc.dram_tensor("bada", [6 * D], F32, kind="ExternalInput")
    out = nc.dram_tensor("out", [D, T], F32, kind="ExternalOutput")

    with tile.TileContext(nc, pool_alloc_mode="queue") as tc:
      for _rep in range(repeat):
        with ExitStack() as X:
            const = X.enter_context(tc.tile_pool(name="const", bufs=1))
            resid = X.enter_context(tc.tile_pool(name="resid", bufs=1))
            dram = X.enter_context(tc.tile_pool(name="dram", bufs=1, space="DRAM"))

            # ---------------- constants + x ----------------
            g_bc = const.tile([P, 4], F32)
            nc.sync.dma_start(out=g_bc, in_=bc_ap(g, 4))
            gsq = const.tile([P, 4], F32)
            nc.vector.tensor_tensor(gsq, g_bc, g_bc, OP.mult)
            ginv2 = const.tile([P, 4], F32)
            nc.vector.reciprocal(ginv2, gsq)
            scl_n1 = const.tile([P, 1], F32)
            nc.vector.tensor_scalar_mul(scl_n1, ginv2[:, 0:1], 1.0 / D)
            scl_n2 = const.tile([P, 1], F32)
            nc.vector.tensor_scalar_mul(scl_n2, ginv2[:, 1:2], 1.0 / D)
            scl_q = const.tile([P, 1], F32)
            nc.vector.tensor_copy(scl_q, ginv2[:, 2:3])
            scl_k = const.tile([P, 1], F32)
            nc.vector.tensor_scalar_mul(scl_k, ginv2[:, 3:4], 1.0 / HD)

            ones_f = const.tile([P, 16], F32)
            nc.gpsimd.memset(ones_f, 1.0)
            onesh_f = const.tile([P, 2], F32)
            nc.gpsimd.memset(onesh_f, 0.0)
            nc.gpsimd.memset(onesh_f[0:64, 0:1], 1.0)
            nc.gpsimd.memset(onesh_f[64:128, 1:2], 1.0)
            ones1 = const.tile([P, 1], mm_dt)
            nc.vector.tensor_copy(ones1, ones_f[:, 0:1])
            onesh = const.tile([P, 2], mm_dt)
            nc.vector.tensor_copy(onesh, onesh_f)

            bqkv_c = const.tile([P, 3 * D // P], F32)
            nc.sync.dma_start(out=bqkv_c, in_=col_ap(bqkv, 3 * D // P))
            bproj_c = const.tile([P, NCH], F32)
            nc.sync.dma_start(out=bproj_c, in_=col_ap(bproj, NCH))
            bfc1_c = const.tile([P, DM // P], F32)
            nc.sync.dma_start(out=bfc1_c, in_=col_ap(bfc1, DM // P))
            bfc2_c = const.tile([P, NCH], F32)
            nc.sync.dma_start(out=bfc2_c, in_=col_ap(bfc2, NCH))
            vbias_bc = const.tile([P, D], F32)
            nc.sync.dma_start(out=vbias_bc, in_=bc_ap(bqkv, D, offset=2 * D))

            x_res = resid.tile([P, NCH, T], F32)
            for j in range(NCH):
                nc.sync.dma_start(out=x_res[:, j, :], in_=xt[j * P:(j + 1) * P, :])

            c_pm = const.tile([P, NCH], F32)
            nc.sync.dma_start(out=c_pm, in_=col_ap(cin, NCH))
            cs_pm = const.tile([P, NCH], BF16)
            nc.scalar.activation(cs_pm, c_pm, AF.Silu)

            # ---------------- adaLN ----------------
            ada_scr = dram.tile([1, 6 * D], F32)
            with tc.tile_pool(name="wadap", bufs=4) as wp, \
                 tc.tile_pool(name="adev", bufs=3) as adev, \
                 tc.tile_pool(name="psA", bufs=2, space="PSUM") as psA:
                for nb in range(6 * D // 512):
                    pa = psA.tile([1, 512], F32, name="pa")
                    for d in range(NCH):
                        wt = wp.tile([P, 512], BF16, name="wt")
                        nc.sync.dma_start(out=wt, in_=wada[d, :, nb * 512:(nb + 1) * 512])
                        nc.tensor.matmul(pa, cs_pm[:, d:d + 1], wt,
                                         start=(d == 0), stop=(d == NCH - 1))
                    stage = adev.tile([1, 512], F32, name="stage")
                    nc.vector.tensor_copy(stage, pa)
                    nc.sync.dma_start(out=ada_scr[:, nb * 512:(nb + 1) * 512], in_=stage)
            adaT = const.tile([P, 48], F32)
            nc.sync.dma_start(out=adaT, in_=bass.AP(tensor=ada_scr.tensor, offset=0,
                                                    ap=[[1, P], [P, 48]]))
            badaT = const.tile([P, 48], F32)
            nc.sync.dma_start(out=badaT, in_=col_ap(bada, 48))
            nc.vector.tensor_tensor(adaT, adaT, badaT, OP.add)
            # cols: shift_msa 0:8 | scale_msa 8:16 | gate_msa 16:24
            #       shift_mlp 24:32 | scale_mlp 32:40 | gate_mlp 40:48
            nc.vector.tensor_scalar_add(adaT[:, 8:16], adaT[:, 8:16], 1.0)
            nc.vector.tensor_scalar_add(adaT[:, 32:40], adaT[:, 32:40], 1.0)
            gb_proj = const.tile([P, NCH], F32)
            nc.vector.tensor_tensor(gb_proj, adaT[:, 16:24], bproj_c, OP.mult)
            gb_fc2 = const.tile([P, NCH], F32)
            nc.vector.tensor_tensor(gb_fc2, adaT[:, 40:48], bfc2_c, OP.mult)

            def norm_modulate(src, scl, sh_col, sc_col, h_out, h_np):
                with tc.tile_pool(name="sqp", bufs=3) as sqp, \
                     tc.tile_pool(name="psN", bufs=2, space="PSUM") as psN, \
                     tc.tile_pool(name="nrm", bufs=1) as nrm:
                    pss = [psN.tile([1, 512], F32, name="pss") for _ in range(2)]
                    for j in range(NCH):
                        xsq = sqp.tile([P, T], h_np, name="xsq")
                        nc.scalar.activation(xsq, src[:, j, :], AF.Square)
                        for t2 in range(2):
                            nc.tensor.matmul(pss[t2], ones1 if h_np == mm_dt else ones1,
                                             xsq[:, t2 * 512:(t2 + 1) * 512],
                                             start=(j == 0), stop=(j == NCH - 1))
                    rr = nrm.tile([1, T], F32, name="rr")
                    for t2 in range(2):
                        nc.scalar.activation(rr[:, t2 * 512:(t2 + 1) * 512], pss[t2],
                                             AF.Sqrt, scale=scl[0:1, :])
                    rinv = nrm.tile([1, T], F32, name="rinv")
                    nc.vector.reciprocal(rinv, rr)
                    rbc = nrm.tile([P, T], F32, name="rbc")
                    nc.gpsimd.partition_broadcast(rbc, rinv)
                    with tc.tile_pool(name="xnp", bufs=3) as xnp:
                        for j in range(NCH):
                            xn = xnp.tile([P, T], F32, name="xn")
                            nc.vector.tensor_tensor(xn, src[:, j, :], rbc, OP.mult)
                            nc.vector.tensor_scalar(h_out[:, j, :], xn,
                                                    adaT[:, sc_col + j:sc_col + j + 1],
                                                    adaT[:, sh_col + j:sh_col + j + 1],
                                                    OP.mult, OP.add)

            oT_dram = dram.tile([D, T], mm_dt)

            att = ExitStack()

            with tc.tile_pool(name="h1p", bufs=1, side="right") as h1p:
                h1 = h1p.tile([P, NCH, T], mm_dt)
                # ------------ norm1 + modulate ------------
                norm_modulate(x_res, scl_n1, 0, 8, h1, mm_dt)

                # ------------ q, k (feature-major) + per-head rmsnorm ------------
                qp_ = att.enter_context(tc.tile_pool(name="qp_", bufs=1))
                kp_ = att.enter_context(tc.tile_pool(name="kp_", bufs=1))
                q_t = qp_.tile([P, NCH, T], mm_dt)
                k_t = kp_.tile([P, NCH, T], mm_dt)
                with tc.tile_pool(name="wqp", bufs=3) as wqp, \
                     tc.tile_pool(name="sqq", bufs=2) as sqq, \
                     tc.tile_pool(name="psD", bufs=3, space="PSUM") as psD, \
                     tc.tile_pool(name="psR", bufs=4, space="PSUM") as psR, \
                     tc.tile_pool(name="nrq", bufs=2) as nrq:
                    for fc in range(16):  # q: 0..7, k: 8..15
                        wt = wqp.tile([P, NCH, P], mm_dt, name="wt")
                        for d in range(NCH):
                            nc.sync.dma_start(out=wt[:, d, :],
                                              in_=wqkv[d, :, fc * P:(fc + 1) * P])
                        ps = [psD.tile([P, 512], F32, name="ps") for _ in range(2)]
                        for d in range(NCH):
                            for nt in range(2):
                                nc.tensor.matmul(ps[nt], wt[:, d, :],
                                                 h1[:, d, nt * 512:(nt + 1) * 512],
                                                 start=(d == 0), stop=(d == NCH - 1))
                        prh = [psR.tile([1, 512], F32, name="prh") for _ in range(4)]
                        for nt in range(2):
                            tgt = q_t if fc < 8 else k_t
                            nc.scalar.activation(tgt[:, fc % 8, nt * 512:(nt + 1) * 512],
                                                 ps[nt], AF.Identity, bias=bqkv_c[:, fc:fc + 1])
                            sq = sqq.tile([P, 512], mm_dt, name="sq")
                            nc.scalar.activation(sq, ps[nt], AF.Square,
                                                 bias=bqkv_c[:, fc:fc + 1])
                            for hf in range(2):
                                nc.tensor.matmul(prh[2 * hf + nt], onesh[:, hf:hf + 1],
                                                 sq, start=True, stop=True)
                        sclqk = scl_q if fc < 8 else scl_k
                        for hf in range(2):
                            rr = nrq.tile([1, T], F32, name="rr")
                            for nt in range(2):
                                nc.scalar.activation(rr[:, nt * 512:(nt + 1) * 512],
                                                     prh[2 * hf + nt], AF.Sqrt,
                                                     scale=sclqk[0:1, :])
                            rinv = nrq.tile([1, T], F32, name="rinv")
                            nc.vector.reciprocal(rinv, rr)
                            rbc = nrq.tile([P, T], F32, name="rbc")
                            nc.gpsimd.partition_broadcast(rbc, rinv)
                            tgt = q_t if fc < 8 else k_t
                            nc.vector.tensor_tensor(
                                tgt[64 * hf:64 * (hf + 1), fc % 8, :],
                                tgt[64 * hf:64 * (hf + 1), fc % 8, :],
                                rbc[64 * hf:64 * (hf + 1), :], OP.mult)

                # ------------ v (token-major, ones-augmented) ------------
                vxp = att.enter_context(tc.tile_pool(name="vxp", bufs=1))
                vx = vxp.tile([P, NCH, H, HD + 1], mm_dt)   # [kt_chunk][head][(v,1)]
                for t8 in range(NCH):
                    nc.vector.tensor_copy(vx[:, t8, :, HD], ones_f)
                with tc.tile_pool(name="wvp", bufs=2) as wvp, \
                     tc.tile_pool(name="psV", bufs=3, space="PSUM") as psV:
                    for nq in range(4):  # quarters of the v feature dim
                        wv = wvp.tile([P, NCH, 256], mm_dt, name="wv")
                        for d in range(NCH):
                            nc.sync.dma_start(
                                out=wv[:, d, :],
                                in_=wqkv[d, :, 2 * D + nq * 256:2 * D + (nq + 1) * 256])
                        for t8 in range(NCH):
                            pv = psV.tile([P, 256], F32, name="pv")
                            for d in range(NCH):
                                nc.tensor.matmul(pv, h1[:, d, t8 * P:(t8 + 1) * P],
                                                 wv[:, d, :],
                                                 start=(d == 0), stop=(d == NCH - 1))
                            nc.vector.tensor_tensor(
                                vx[:, t8, nq * 4:(nq + 1) * 4, 0:HD],
                                pv.rearrange("p (h e) -> p h e", e=HD),
                                vbias_bc[:, nq * 256:(nq + 1) * 256].rearrange(
                                    "p (h e) -> p h e", e=HD),
                                OP.add)

            att2 = ExitStack()
            wpjp = att2.enter_context(tc.tile_pool(name="wpjp", bufs=1, side="right"))
            wpj = wpjp.tile([P, NCH, 512], mm_dt)
            for d in range(NCH):
                nc.sync.dma_start(out=wpj[:, d, :], in_=wproj[d, :, 0:512])

            # ------------ attention ------------
            with tc.tile_pool(name="esp", bufs=6) as esp, \
                 tc.tile_pool(name="psS", bufs=2, space="PSUM") as psS, \
                 tc.tile_pool(name="psO", bufs=4, space="PSUM") as psO, \
                 tc.tile_pool(name="onp", bufs=2) as onp:
                for h in range(H):
                    hc, hf = h // 2, h % 2
                    rq = slice(64 * hf, 64 * (hf + 1))
                    es = []
                    for ktc in range(NCH):
                        psc = psS.tile([P, 1024], F32, name="psc")
                        for qt2 in range(2):
                            nc.tensor.matmul(psc[:, qt2 * 512:(qt2 + 1) * 512],
                                             k_t[rq, hc, ktc * P:(ktc + 1) * P],
                                             q_t[rq, hc, qt2 * 512:(qt2 + 1) * 512],
                                             start=True, stop=True)
                        e = esp.tile([P, 1024], mm_dt, name="es")
                        nc.scalar.activation(e, psc, AF.Exp)
                        es.append(e)
                    for qt in range(2):
                        qs = slice(qt * 512, (qt + 1) * 512)
                        po = psO.tile([P, 512], F32, name="po")
                        for ktc in range(NCH):
                            nc.tensor.matmul(po[0:65, :], vx[:, ktc, h, :],
                                             es[ktc][:, qs],
                                             start=(ktc == 0), stop=(ktc == NCH - 1))
                        setmp = onp.tile([128, 512], F32, name="setmp")
                        nc.vector.tensor_copy(setmp[64:65, :], po[64:65, :])
                        se = onp.tile([1, 512], F32, name="se")
                        nc.sync.dma_start(out=se, in_=setmp[64:65, :])
                        rs = onp.tile([1, 512], F32, name="rs")
                        nc.vector.reciprocal(rs, se)
                        rsb = onp.tile([64, 512], F32, name="rsb")
                        nc.gpsimd.partition_broadcast(rsb, rs)
                        on = onp.tile([64, 512], mm_dt, name="on")
                        nc.vector.tensor_tensor(on, po[0:64, :], rsb, OP.mult)
                        nc.sync.dma_start(
                            out=oT_dram[h * HD:(h + 1) * HD, qs], in_=on)

            att.close()  # free vx + qk
            # ------------ proj + residual ------------
            with tc.tile_pool(name="otp", bufs=1) as otp, \
                 tc.tile_pool(name="wpp", bufs=2) as wpp, \
                 tc.tile_pool(name="psP", bufs=4, space="PSUM") as psP, \
                 tc.tile_pool(name="evp", bufs=3) as evp:
                oT_sb = otp.tile([P, NCH, T], mm_dt, bufs=1, name="oT_sb")
                for d in range(NCH):
                    nc.sync.dma_start(out=oT_sb[:, d, :], in_=oT_dram[d * P:(d + 1) * P, :])
                for fcb in range(4):
                    if fcb >= 2:
                        wt = wpp.tile([P, NCH, 256], mm_dt, name="wt")
                        for d in range(NCH):
                            nc.sync.dma_start(out=wt[:, d, :],
                                              in_=wproj[d, :, fcb * 256:(fcb + 1) * 256])
                    ps = [psP.tile([P, 512], F32, name="ps") for _ in range(4)]
                    for d in range(NCH):
                        for fi in range(2):
                            for nt in range(2):
                                lhs = (wpj[:, d, fcb * 256 + fi * P:fcb * 256 + (fi + 1) * P]
                                       if fcb < 2 else wt[:, d, fi * P:(fi + 1) * P])
                                nc.tensor.matmul(ps[2 * fi + nt], lhs,
                                                 oT_sb[:, d, nt * 512:(nt + 1) * 512],
                                                 start=(d == 0), stop=(d == NCH - 1))
                    for fi in range(2):
                        fc = fcb * 2 + fi
                        for nt in range(2):
                            tmp = evp.tile([P, 512], F32, name="tmp")
                            nc.scalar.activation(tmp, ps[2 * fi + nt], AF.Identity,
                                                 scale=adaT[:, 16 + fc:17 + fc],
                                                 bias=gb_proj[:, fc:fc + 1])
                            nc.vector.tensor_tensor(x_res[:, fc, nt * 512:(nt + 1) * 512],
                                                    x_res[:, fc, nt * 512:(nt + 1) * 512],
                                                    tmp, OP.add)

            att2.close()  # free wpj

            # ------------ norm2 + modulate + MLP (token-halved) ------------
            with tc.tile_pool(name="h2p", bufs=1) as h2p:
                h2 = h2p.tile([P, NCH, T], mlp_dt)
                norm_modulate(x_res, scl_n2, 24, 32, h2, mlp_dt)
                for th in range(2):
                    ts_ = slice(th * 512, (th + 1) * 512)
                    with tc.tile_pool(name="gactp", bufs=1) as gactp, \
                         tc.tile_pool(name="wf1p", bufs=4) as wf1p, \
                         tc.tile_pool(name="wf2p", bufs=8) as wf2p, \
                         tc.tile_pool(name="psM", bufs=3, space="PSUM") as psM, \
                         tc.tile_pool(name="psM2", bufs=3, space="PSUM") as psM2, \
                         tc.tile_pool(name="evm", bufs=3) as evm:
                        gact = gactp.tile([P, DM // P, 512], mlp_dt)
                        for fcb in range(16):
                            wt = wf1p.tile([P, NCH, 256], mlp_dt, name="wt")
                            for d in range(NCH):
                                nc.sync.dma_start(out=wt[:, d, :],
                                                  in_=wfc1[d, :, fcb * 256:(fcb + 1) * 256])
                            ps = [psM.tile([P, 512], F32, name="ps") for _ in range(2)]
                            for d in range(NCH):
                                for fi in range(2):
                                    nc.tensor.matmul(ps[fi], wt[:, d, fi * P:(fi + 1) * P],
                                                     h2[:, d, ts_],
                                                     start=(d == 0), stop=(d == NCH - 1))
                            for fi in range(2):
                                fc = fcb * 2 + fi
                                nc.scalar.activation(gact[:, fc, :], ps[fi],
                                                     AF.Gelu_apprx_tanh,
                                                     bias=bfc1_c[:, fc:fc + 1])
                        for fcb in range(4):
                            ps2 = [psM2.tile([P, 512], F32, name="ps2") for _ in range(2)]
                            for d32 in range(DM // P):
                                wt2 = wf2p.tile([P, 256], mlp_dt, name="wt2")
                                nc.sync.dma_start(out=wt2,
                                                  in_=wfc2[d32, :, fcb * 256:(fcb + 1) * 256])
                                for fi in range(2):
                                    nc.tensor.matmul(ps2[fi], wt2[:, fi * P:(fi + 1) * P],
                                                     gact[:, d32, :],
                                                     start=(d32 == 0),
                                                     stop=(d32 == DM // P - 1))
                            for fi in range(2):
                                fc = fcb * 2 + fi
                                tmp = evm.tile([P, 512], F32, name="tmp")
                                nc.scalar.activation(tmp, ps2[fi], AF.Identity,
                                                     scale=adaT[:, 40 + fc:41 + fc],
                                                     bias=gb_fc2[:, fc:fc + 1])
                                ot = evm.tile([P, 512], F32, name="otout")
                                nc.vector.tensor_tensor(ot, x_res[:, fc, ts_], tmp, OP.add)
                                nc.sync.dma_start(out=out[fc * P:(fc + 1) * P, ts_], in_=ot)
    nc.compile()
    return nc


_CACHE = {}


def _runner(nc, n_cores=8):
    import jax
    import numpy as _np
    from jax.sharding import Mesh, PartitionSpec, NamedSharding
    from jax.experimental.shard_map import shard_map
    from concourse.bass2jax import _bass_exec_p, install_neuronx_cc_hook, partition_id_tensor

    install_neuronx_cc_hook()
    in_names, out_names, out_avals = [], [], []
    partition_name = nc.partition_id_tensor.name if nc.partition_id_tensor else None
    for alloc in nc.m.functions[0].allocations:
        if not isinstance(alloc, mybir.MemoryLocationSet):
            continue
        nm = alloc.memorylocations[0].name
        if alloc.kind == "ExternalInput":
            if nm != partition_name:
                in_names.append(nm)
        elif alloc.kind == "ExternalOutput":
            out_names.append(nm)
            out_avals.append(jax.core.ShapedArray(tuple(alloc.tensor_shape),
                                                  mybir.dt.np(alloc.dtype)))

    def _body(*args):
        operands = list(args)
        if partition_name is not None:
            operands.append(partition_id_tensor())
        outs = _bass_exec_p.bind(
            *operands,
            out_avals=tuple(out_avals),
            in_names=tuple(in_names + [partition_name] if partition_name else in_names),
            out_names=tuple(out_names),
            lowering_input_output_aliases=(),
            sim_require_finite=False,
            sim_require_nnan=False,
            nc=nc,
        )
        return tuple(outs)

    devices = jax.devices()[:n_cores]
    mesh = Mesh(_np.asarray(devices), ("core",))
    fn = jax.jit(shard_map(_body, mesh=mesh,
                           in_specs=(PartitionSpec("core"),) * len(in_names),
                           out_specs=(PartitionSpec("core"),) * len(out_names),
                           check_rep=False))

    def run(in_maps):
        concat = [_np.concatenate([_np.asarray(m[n]) for m in in_maps], axis=0)
                  for n in in_names]
        args = [jax.device_put(c, NamedSharding(mesh, PartitionSpec("core")))
                for c in concat]
        outs = fn(*args)
        jax.block_until_ready(outs)
        res = []
        for c in range(n_cores):
            d = {}
            for i, nm in enumerate(out_names):
                full = _np.asarray(outs[i])
                d[nm] = full.reshape(n_cores, *out_avals[i].shape)[c]
            res.append(d)
        return res

    return run


def kernel(**inputs):
    """Full (unsharded) inputs -> full (B, T, D) float32 output."""
    if "nc" not in _CACHE:
        _CACHE["nc"] = build_dit(n_cores=8)
        _CACHE["run"] = _runner(_CACHE["nc"], 8)
    in_maps = host_prep(**inputs)
    results = _CACHE["run"](in_maps)
    return host_post(results)



# revision 19
# speedup vs baseline: 1.9704x; 1.9704x over previous
"""nn_DiTBlock on 8 TRN2 NeuronCores: data-parallel over batch (B=8), one
batch element per core. Self-contained: builds the Bass/Tile kernel, shards
inputs on the host (transpose/pack/cast only), runs SPMD via bass2jax/PJRT,
gathers and un-transposes the output.

v2 design: fp8e4(e4m3)+DoubleRow matmuls for qkv/v/attn@v/proj/fc1/fc2
(weights host-prescaled x128, descale folded into evictions), bf16 for adaLN
and attention scores, f32 residual + PSUM. exp scaled by 1/32 (cancels in
softmax); k-rmsnorm folded into the exp's per-partition scale; odd heads'
attn@v written directly to PSUM partitions 64:128 (no partition-move DMAs);
single-pass MLP; multi-chunk batched weight DMAs."""

import numpy as np
from contextlib import ExitStack

import concourse.bass as bass
import concourse.mybir as mybir
import concourse.tile as tile
from concourse import bacc


F32 = mybir.dt.float32
F32R = mybir.dt.float32r
BF16 = mybir.dt.bfloat16
FP8 = mybir.dt.float8e4
AF = mybir.ActivationFunctionType
OP = mybir.AluOpType
DR = mybir.MatmulPerfMode.DoubleRow

B, T, D, H = 8, 1024, 1024, 16
HD = D // H          # 64
DM = 4 * D           # 4096
NCH = D // 128       # 8
MCH = DM // 128      # 32
P = 128
WS = 128.0           # fp8 weight pre-scale (host)
ISV = 1.0 / WS
ELN32 = -3.4657359027997265  # -ln(32): exp pre-scale so fp8 es stays < 240


def host_prep(x, c, g1, g2, gq, gk, Wqkv, bqkv, Wproj, bproj,
              Wfc1, bfc1, Wfc2, bfc2, Wada, bada):
    import ml_dtypes
    E4 = mybir.dt.np(FP8)

    def packT(W, npdt, scale=1.0):  # (F, K) -> (K//128, 128, F) contiguous
        Wt = np.ascontiguousarray(np.asarray(W, np.float32).T * scale).astype(npdt)
        K, F = Wt.shape
        return np.ascontiguousarray(Wt.reshape(K // 128, 128, F))

    f32 = np.float32
    com = {
        "wqkv": packT(Wqkv, E4, WS), "wproj": packT(Wproj, E4, WS),
        "wfc1": packT(Wfc1, E4, WS), "wfc2": packT(Wfc2, E4, WS),
        "wada": packT(Wada, ml_dtypes.bfloat16),
        "bqkv": np.asarray(bqkv, f32), "bproj": np.asarray(bproj, f32),
        "bfc1": np.asarray(bfc1, f32), "bfc2": np.asarray(bfc2, f32),
        "bada": np.asarray(bada, f32),
        "g": np.stack([np.asarray(g1)[0], np.asarray(g2)[0],
                       np.asarray(gq)[0], np.asarray(gk)[0]]).astype(f32),
    }
    in_maps = []
    for b in range(B):
        m = dict(com)
        m["xt"] = np.ascontiguousarray(np.asarray(x[b], f32).T)
        m["cvec"] = np.asarray(c[b], f32)
        in_maps.append(m)
    return in_maps


def host_post(results):
    return np.ascontiguousarray(
        np.stack([r["out"].T for r in results]).astype(np.float32))


def col_ap(handle, nch):
    """DRAM (nch*128,) viewed as [128, nch]: tile[p, ch] = v[ch*128+p]."""
    return bass.AP(tensor=handle, offset=0, ap=[[1, P], [P, nch]])


def bc_ap(handle, n, offset=0):
    """DRAM (n,) broadcast-read to [128, n] (partition stride 0)."""
    return bass.AP(tensor=handle, offset=offset, ap=[[0, P], [1, n]])


def wload_ap(handle, kch, cols, col0):
    """DRAM weight pack [KCH,128,F] -> [128, kch, cols] AP at col offset."""
    F = handle.shape[2]
    return bass.AP(tensor=handle, offset=col0,
                   ap=[[F, P], [P * F, kch], [1, cols]])


def build_dit(n_cores=8):
    nc = bacc.Bacc("TRN2", target_bir_lowering=False, debug=False,
                   num_devices=n_cores)

    xt = nc.dram_tensor("xt", [D, T], F32, kind="ExternalInput")
    cin = nc.dram_tensor("cvec", [D], F32, kind="ExternalInput")
    g = nc.dram_tensor("g", [4], F32, kind="ExternalInput")
    wqkv = nc.dram_tensor("wqkv", [NCH, P, 3 * D], FP8, kind="ExternalInput")
    wproj = nc.dram_tensor("wproj", [NCH, P, D], FP8, kind="ExternalInput")
    wfc1 = nc.dram_tensor("wfc1", [NCH, P, DM], FP8, kind="ExternalInput")
    wfc2 = nc.dram_tensor("wfc2", [MCH, P, D], FP8, kind="ExternalInput")
    wada = nc.dram_tensor("wada", [NCH, P, 6 * D], BF16, kind="ExternalInput")
    bqkv = nc.dram_tensor("bqkv", [3 * D], F32, kind="ExternalInput")
    bproj = nc.dram_tensor("bproj", [D], F32, kind="ExternalInput")
    bfc1 = nc.dram_tensor("bfc1", [DM], F32, kind="ExternalInput")
    bfc2 = nc.dram_tensor("bfc2", [D], F32, kind="ExternalInput")
    bada = nc.dram_tensor("bada", [6 * D], F32, kind="ExternalInput")
    out = nc.dram_tensor("out", [D, T], F32, kind="ExternalOutput")

    with tile.TileContext(nc, pool_alloc_mode="queue") as tc:
        with ExitStack() as X:
            const = X.enter_context(tc.tile_pool(name="const", bufs=1))
            resid = X.enter_context(tc.tile_pool(name="resid", bufs=1))
            dram = X.enter_context(tc.tile_pool(name="dram", bufs=1, space="DRAM"))

            # ---------------- constants ----------------
            g_bc = const.tile([P, 4], F32)
            nc.sync.dma_start(out=g_bc, in_=bc_ap(g, 4))
            gsq = const.tile([P, 4], F32)
            nc.vector.tensor_tensor(gsq, g_bc, g_bc, OP.mult)
            ginv2 = const.tile([P, 4], F32)
            nc.vector.reciprocal(ginv2, gsq)
            # Rsqrt scales: rinv = rsqrt(ps * scl)
            scl_n1 = const.tile([P, 1], F32)
            nc.vector.tensor_scalar_mul(scl_n1, ginv2[:, 0:1], 1.0 / D)
            scl_n2 = const.tile([P, 1], F32)
            nc.vector.tensor_scalar_mul(scl_n2, ginv2[:, 1:2], 1.0 / D)
            scl_q = const.tile([P, 1], F32)
            nc.vector.tensor_copy(scl_q, ginv2[:, 2:3])
            scl_k = const.tile([P, 1], F32)
            nc.vector.tensor_scalar_mul(scl_k, ginv2[:, 3:4], 1.0 / HD)

            ones1_f = const.tile([P, 1], F32)
            nc.gpsimd.memset(ones1_f, 1.0)
            ones1 = const.tile([P, 1], BF16)
            nc.vector.tensor_copy(ones1, ones1_f)
            onesh_f = const.tile([P, 2], F32)
            nc.gpsimd.memset(onesh_f, 0.0)
            nc.gpsimd.memset(onesh_f[0:64, 0:1], 1.0)
            nc.gpsimd.memset(onesh_f[64:128, 1:2], 1.0)
            onesh = const.tile([P, 2], BF16)
            nc.vector.tensor_copy(onesh, onesh_f)

            bqkv_c = const.tile([P, 3 * D // P], F32)
            nc.sync.dma_start(out=bqkv_c, in_=col_ap(bqkv, 3 * D // P))
            bproj_c = const.tile([P, NCH], F32)
            nc.sync.dma_start(out=bproj_c, in_=col_ap(bproj, NCH))
            bfc1_c = const.tile([P, MCH], F32)
            nc.sync.dma_start(out=bfc1_c, in_=col_ap(bfc1, MCH))
            bfc2_c = const.tile([P, NCH], F32)
            nc.sync.dma_start(out=bfc2_c, in_=col_ap(bfc2, NCH))
            vbias_bc = const.tile([P, D], F32)
            nc.sync.dma_start(out=vbias_bc, in_=bc_ap(bqkv, D, offset=2 * D))
            eln32_c = const.tile([P, 1], F32)
            nc.gpsimd.memset(eln32_c, ELN32)

            x_res = resid.tile([P, NCH, T], F32)
            for j in range(4):
                nc.sync.dma_start(
                    out=x_res[:, 2 * j:2 * j + 2, :],
                    in_=bass.AP(tensor=xt, offset=2 * j * P * T,
                                ap=[[T, P], [P * T, 2], [1, T]]))

            c_pm = const.tile([P, NCH], F32)
            nc.sync.dma_start(out=c_pm, in_=col_ap(cin, NCH))
            cs_pm = const.tile([P, NCH], BF16)
            nc.scalar.activation(cs_pm, c_pm, AF.Silu)

            # ---------------- adaLN (bf16) ----------------
            ada_scr = dram.tile([1, 6 * D], F32)
            ada_sb = const.tile([1, 6 * D], F32)
            with tc.tile_pool(name="wadap", bufs=3) as wp, \
                 tc.tile_pool(name="psA", bufs=2, space="PSUM") as psA:
                for nb in range(12):
                    wt = wp.tile([P, NCH, 512], BF16, name="wt")
                    nc.sync.dma_start(out=wt, in_=wload_ap(wada, NCH, 512, nb * 512))
                    pa = psA.tile([1, 512], F32, name="pa")
                    for d in range(NCH):
                        nc.tensor.matmul(pa, cs_pm[:, d:d + 1], wt[:, d, :],
                                         start=(d == 0), stop=(d == NCH - 1))
                    nc.vector.tensor_copy(ada_sb[:, nb * 512:(nb + 1) * 512], pa)
            nc.sync.dma_start(out=ada_scr, in_=ada_sb)
            adaT = const.tile([P, 48], F32)
            nc.sync.dma_start(out=adaT, in_=bass.AP(tensor=ada_scr.tensor, offset=0,
                                                    ap=[[1, P], [P, 48]]))
            badaT = const.tile([P, 48], F32)
            nc.sync.dma_start(out=badaT, in_=col_ap(bada, 48))
            nc.vector.tensor_tensor(adaT, adaT, badaT, OP.add)
            # cols: shift_msa 0:8 | scale_msa 8:16 | gate_msa 16:24
            #       shift_mlp 24:32 | scale_mlp 32:40 | gate_mlp 40:48
            nc.vector.tensor_scalar_add(adaT[:, 8:16], adaT[:, 8:16], 1.0)
            nc.vector.tensor_scalar_add(adaT[:, 32:40], adaT[:, 32:40], 1.0)
            gb_proj = const.tile([P, NCH], F32)
            nc.vector.tensor_tensor(gb_proj, adaT[:, 16:24], bproj_c, OP.mult)
            gbs_proj = const.tile([P, NCH], F32)
            nc.vector.tensor_scalar_mul(gbs_proj, adaT[:, 16:24], ISV)
            gb_fc2 = const.tile([P, NCH], F32)
            nc.vector.tensor_tensor(gb_fc2, adaT[:, 40:48], bfc2_c, OP.mult)
            gbs_fc2 = const.tile([P, NCH], F32)
            nc.vector.tensor_scalar_mul(gbs_fc2, adaT[:, 40:48], ISV)

            def norm_modulate(scl, sh_col, sc_col, h_out):
                """x_res (f32) -> h_out (fp8): rmsnorm + adaLN modulate."""
                with tc.tile_pool(name="sqp", bufs=3) as sqp, \
                     tc.tile_pool(name="psN", bufs=1, space="PSUM") as psN, \
                     tc.tile_pool(name="nrm", bufs=1) as nrm:
                    pss = psN.tile([1, T], F32, name="pss")
                    for j in range(NCH):
                        xsq = sqp.tile([P, T], BF16, name="xsq")
                        nc.vector.tensor_tensor(xsq, x_res[:, j, :],
                                                x_res[:, j, :], OP.mult)
                        for t2 in range(2):
                            nc.tensor.matmul(pss[:, t2 * 512:(t2 + 1) * 512],
                                             ones1, xsq[:, t2 * 512:(t2 + 1) * 512],
                                             start=(j == 0), stop=(j == NCH - 1))
                    rr = nrm.tile([1, T], F32, name="rr")
                    for t2 in range(2):
                        nc.scalar.activation(rr[:, t2 * 512:(t2 + 1) * 512],
                                             pss[:, t2 * 512:(t2 + 1) * 512],
                                             AF.Sqrt, scale=scl[0:1, :])
                    rinv = nrm.tile([1, T], F32, name="rinv")
                    nc.vector.reciprocal(rinv, rr)
                    rbc = nrm.tile([P, T], F32, name="rbc")
                    nc.gpsimd.partition_broadcast(rbc, rinv)
                    with tc.tile_pool(name="xnp", bufs=3) as xnp:
                        for j in range(NCH):
                            xn = xnp.tile([P, T], F32, name="xn")
                            nc.vector.tensor_tensor(xn, x_res[:, j, :], rbc, OP.mult)
                            nc.gpsimd.tensor_scalar(h_out[:, j, :], xn,
                                                    adaT[:, sc_col + j:sc_col + j + 1],
                                                    adaT[:, sh_col + j:sh_col + j + 1],
                                                    OP.mult, OP.add)

            att = ExitStack()
            h1p = att.enter_context(tc.tile_pool(name="h1p", bufs=1, side="right"))
            h1 = h1p.tile([P, NCH, T], FP8)
            # ------------ norm1 + modulate ------------
            norm_modulate(scl_n1, 0, 8, h1)

            # ------------ q, k (feature-major bf16) + per-head rmsnorm ------------
            qp_ = att.enter_context(tc.tile_pool(name="qp_", bufs=1))
            kp_ = att.enter_context(tc.tile_pool(name="kp_", bufs=1))
            q_t = qp_.tile([P, NCH, T], BF16)
            k_t = kp_.tile([P, NCH, T], BF16)
            rkcp = att.enter_context(tc.tile_pool(name="rkcp", bufs=1))
            rkc = rkcp.tile([P, H, NCH], F32)  # 1/|k| per k-token, head-major

            with tc.tile_pool(name="wqp", bufs=2) as wqp, \
                 tc.tile_pool(name="sqq", bufs=2) as sqq, \
                 tc.tile_pool(name="psD", bufs=2, space="PSUM") as psD, \
                 tc.tile_pool(name="psR", bufs=1, space="PSUM") as psR, \
                 tc.tile_pool(name="nrq", bufs=2) as nrq:
                for fc in range(16):  # q: 0..7, k: 8..15
                    if fc % 4 == 0:
                        wt = wqp.tile([P, NCH, 512], FP8, name="wt")
                        nc.sync.dma_start(out=wt, in_=wload_ap(wqkv, NCH, 512, fc * P))
                    tgt = q_t if fc < 8 else k_t
                    ch = fc % 8
                    ps = [psD.tile([P, 512], F32, name="ps") for _ in range(2)]
                    for nt in range(2):
                        for dp in range(4):
                            nc.tensor.matmul(
                                ps[nt],
                                wt[:, 2 * dp:2 * dp + 2, (fc % 4) * P:(fc % 4 + 1) * P],
                                h1[:, 2 * dp:2 * dp + 2, nt * 512:(nt + 1) * 512],
                                start=(dp == 0), stop=(dp == 3), perf_mode=DR)
                        # evict: (psum/128 + bias) -> bf16 (DVE: Pool can't
                        # read PSUM on hardware)
                        nc.vector.tensor_scalar(tgt[:, ch, nt * 512:(nt + 1) * 512],
                                                ps[nt], ISV, bqkv_c[:, fc:fc + 1],
                                                OP.mult, OP.add)
                    # sum of squares per head
                    sq = sqq.tile([P, T], BF16, name="sq")
                    nc.vector.tensor_tensor(sq, tgt[:, ch, :], tgt[:, ch, :], OP.mult)
                    if fc < 8:
                        # q: per-half sums, each in its own row-0 psum tile;
                        # broadcasts always source partition 0 into full tiles
                        for hfq in range(2):
                            prh = psR.tile([1, T], F32, name=f"prh{hfq}")
                            for nt in range(2):
                                nc.tensor.matmul(
                                    prh[:, nt * 512:(nt + 1) * 512],
                                    onesh[:, hfq:hfq + 1],
                                    sq[:, nt * 512:(nt + 1) * 512],
                                    start=True, stop=True)
                            rr2 = nrq.tile([1, T], BF16, name=f"rr2{hfq}")
                            nc.scalar.activation(rr2, prh, AF.Sqrt,
                                                 scale=scl_q[0:1, :])
                            rinv_sb = nrq.tile([1, T], BF16, name=f"ri{hfq}")
                            with nc.allow_low_precision(reason="1/|q| bf16"):
                                nc.vector.reciprocal(rinv_sb, rr2)
                            rbcq = nrq.tile([P, T], BF16, name=f"rbcq{hfq}")
                            nc.gpsimd.partition_broadcast(rbcq, rinv_sb)
                            hs = slice(64 * hfq, 64 * (hfq + 1))
                            nc.vector.tensor_tensor(q_t[hs, ch, :], q_t[hs, ch, :],
                                                    rbcq[hs, :], OP.mult)
                    else:
                        # k: sums token-major [128, NCH] per head -> 1/|k| into
                        # rkc, consumed as the exp's per-partition scale.
                        for j in range(2):
                            hidx = 2 * (fc - 8) + j
                            pkn = psR.tile([P, NCH], F32, name="pkn")
                            for kt in range(NCH):
                                nc.tensor.matmul(
                                    pkn[:, kt:kt + 1],
                                    sq[64 * j:64 * (j + 1), kt * P:(kt + 1) * P],
                                    ones1[64 * j:64 * (j + 1), :],
                                    start=True, stop=True)
                            rrk = nrq.tile([P, NCH], F32, name="rrk")
                            nc.scalar.activation(rrk, pkn, AF.Sqrt, scale=scl_k)
                            nc.vector.reciprocal(rkc[:, hidx, :], rrk)

            # ------------ v (token-major fp8, ones-augmented) ------------
            # vx per-head 128-col slot: even h = [v(0:64) | ones@64 | 0],
            # odd h = [0 | ones@63 | v(64:128)]; attn@v DR outputs are then
            # always full [128, N] (walrus requires that) and odd heads land
            # on PSUM partitions 64:128 directly.
            vxp = att.enter_context(tc.tile_pool(name="vxp", bufs=1))
            vx = vxp.tile([P, NCH, H, P], FP8)   # [ktok][ktc][head][col]
            nc.gpsimd.memset(vx, 0.0)
            for h in range(H):
                oc = HD if h % 2 == 0 else 0
                nc.gpsimd.memset(vx[:, :, h, oc:oc + 1], 1.0)
            with tc.tile_pool(name="wvp", bufs=2) as wvp, \
                 tc.tile_pool(name="psV", bufs=3, space="PSUM") as psV:
                for nq in range(2):
                    wv = wvp.tile([P, NCH, 512], FP8, name="wv")
                    nc.sync.dma_start(out=wv,
                                      in_=wload_ap(wqkv, NCH, 512, 2 * D + nq * 512))
                    for t8 in range(NCH):
                        pv = psV.tile([P, 512], F32, name="pv")
                        for dp in range(4):
                            nc.tensor.matmul(
                                pv, h1[:, 2 * dp:2 * dp + 2, t8 * P:(t8 + 1) * P],
                                wv[:, 2 * dp:2 * dp + 2, :],
                                start=(dp == 0), stop=(dp == 3), perf_mode=DR)
                        # heads alternate col-base 0 (even) / 64 (odd) in vx
                        vblk = vx[:, t8, :, :].rearrange(
                            "p h c -> p (h c)").rearrange(
                            "p (i r) -> p i r", r=256)  # [P, 8, 256]
                        for par in range(2):
                            nc.vector.scalar_tensor_tensor(
                                vblk[:, 4 * nq:4 * nq + 4,
                                     192 * par:192 * par + HD],
                                pv.rearrange("p (i r) -> p i r", r=128)[
                                    :, :, par * HD:(par + 1) * HD], ISV,
                                vbias_bc[:, nq * 512:(nq + 1) * 512].rearrange(
                                    "p (i r) -> p i r", r=128)[
                                    :, :, par * HD:(par + 1) * HD],
                                OP.mult, OP.add)

            # ------------ attention ------------
            oTp = att.enter_context(tc.tile_pool(name="oTp", bufs=1, side="right"))
            oT = oTp.tile([P, NCH, T], FP8)
            with tc.tile_pool(name="esp", bufs=2) as esp, \
                 tc.tile_pool(name="psS", bufs=2, space="PSUM") as psS, \
                 tc.tile_pool(name="psO", bufs=2, space="PSUM") as psO, \
                 tc.tile_pool(name="onp", bufs=4) as onp:
                for h in range(H):
                    hc, hf = h // 2, h % 2
                    rq = slice(64 * hf, 64 * (hf + 1))
                    es_h = esp.tile([P, NCH, T], FP8, name="es")
                    for ktc in range(NCH):
                        psc = psS.tile([P, T], F32, name="psc")
                        for qt in range(2):
                            nc.tensor.matmul(psc[:, qt * 512:(qt + 1) * 512],
                                             k_t[rq, hc, ktc * P:(ktc + 1) * P],
                                             q_t[rq, hc, qt * 512:(qt + 1) * 512],
                                             start=True, stop=True)
                        nc.scalar.activation(es_h[:, ktc, :], psc, AF.Exp,
                                             bias=eln32_c, scale=rkc[:, h, ktc:ktc + 1])
                    for qt in range(2):
                        qs = slice(qt * 512, (qt + 1) * 512)
                        po = psO.tile([P, 512], F32, name="po")
                        rs = onp.tile([P, 512], F32, name="rs")
                        rsb = onp.tile([P, 512], F32, name="rsb")
                        for kp in range(4):
                            nc.tensor.matmul(
                                po, vx[:, 2 * kp:2 * kp + 2, h, :],
                                es_h[:, 2 * kp:2 * kp + 2, qs],
                                start=(kp == 0), stop=(kp == 3), perf_mode=DR)
                        if hf == 0:
                            # denom at row 64: recip there, DMA row to
                            # partition 0, broadcast full, use rows 0:64
                            nc.vector.reciprocal(rs[64:65, :], po[64:65, :])
                            rse = onp.tile([1, 512], F32, name="rse")
                            nc.sync.dma_start(out=rse, in_=rs[64:65, :])
                            nc.gpsimd.partition_broadcast(rsb, rse)
                            nc.vector.tensor_tensor(oT[0:64, hc, qs], po[0:64, :],
                                                    rsb[0:64, :], OP.mult)
                        else:
                            # denom at row 0: broadcast full, use rows 64:128
                            nc.vector.reciprocal(rs[0:1, :], po[0:1, :])
                            nc.gpsimd.partition_broadcast(rsb, rs[0:1, :])
                            nc.vector.tensor_tensor(oT[64:128, hc, qs], po[64:128, :],
                                                    rsb[64:128, :], OP.mult)

            # ------------ proj + residual ------------
            with tc.tile_pool(name="wpp", bufs=2) as wpp, \
                 tc.tile_pool(name="psP", bufs=3, space="PSUM") as psP:
                for half in range(2):
                    wpj = wpp.tile([P, NCH, 512], FP8, name="wpj")
                    nc.sync.dma_start(out=wpj, in_=wload_ap(wproj, NCH, 512, half * 512))
                    for fi in range(4):
                        fc = half * 4 + fi
                        for nt in range(2):
                            pp = psP.tile([P, 512], F32, name="pp")
                            for dp in range(4):
                                nc.tensor.matmul(
                                    pp, wpj[:, 2 * dp:2 * dp + 2, fi * P:(fi + 1) * P],
                                    oT[:, 2 * dp:2 * dp + 2, nt * 512:(nt + 1) * 512],
                                    start=(dp == 0), stop=(dp == 3), perf_mode=DR)
                            nc.vector.affine_then_add(
                                x_res[:, fc, nt * 512:(nt + 1) * 512], pp,
                                x_res[:, fc, nt * 512:(nt + 1) * 512],
                                scale=gbs_proj[:, fc:fc + 1],
                                bias=gb_proj[:, fc:fc + 1])

            att.close()  # free h1, q/k, vx, oT, rkc

            # ------------ norm2 + modulate + MLP (single pass, fp8) ------------
            with tc.tile_pool(name="h2p", bufs=1) as h2p, \
                 tc.tile_pool(name="gactp", bufs=1, side="right") as gactp:
                h2 = h2p.tile([P, NCH, T], FP8)
                norm_modulate(scl_n2, 24, 32, h2)
                gact = gactp.tile([P, MCH, T], FP8)
                with tc.tile_pool(name="wf1p", bufs=2) as wf1p, \
                     tc.tile_pool(name="psM", bufs=2, space="PSUM") as psM:
                    for mg in range(8):
                        w1 = wf1p.tile([P, NCH, 512], FP8, name="w1")
                        nc.sync.dma_start(out=w1, in_=wload_ap(wfc1, NCH, 512, mg * 512))
                        for mi in range(4):
                            m = mg * 4 + mi
                            psm = psM.tile([P, T], F32, name="psm")
                            for nt in range(2):
                                for dp in range(4):
                                    nc.tensor.matmul(
                                        psm[:, nt * 512:(nt + 1) * 512],
                                        w1[:, 2 * dp:2 * dp + 2, mi * P:(mi + 1) * P],
                                        h2[:, 2 * dp:2 * dp + 2, nt * 512:(nt + 1) * 512],
                                        start=(dp == 0), stop=(dp == 3), perf_mode=DR)
                            nc.scalar.activation(gact[:, m, :], psm,
                                                 AF.Gelu_apprx_tanh, scale=ISV,
                                                 bias=bfc1_c[:, m:m + 1])
                with tc.tile_pool(name="wf2p", bufs=2) as wf2p, \
                     tc.tile_pool(name="psM2", bufs=3, space="PSUM") as psM2:
                    for half in range(2):
                        w2 = wf2p.tile([P, MCH, 512], FP8, name="w2")
                        nc.sync.dma_start(out=w2, in_=wload_ap(wfc2, MCH, 512, half * 512))
                        for fi in range(4):
                            fc = half * 4 + fi
                            for nt in range(2):
                                ps2 = psM2.tile([P, 512], F32, name="ps2")
                                for dp in range(16):
                                    nc.tensor.matmul(
                                        ps2,
                                        w2[:, 2 * dp:2 * dp + 2, fi * P:(fi + 1) * P],
                                        gact[:, 2 * dp:2 * dp + 2,
                                             nt * 512:(nt + 1) * 512],
                                        start=(dp == 0), stop=(dp == 15), perf_mode=DR)
                                nc.vector.affine_then_add(
                                    x_res[:, fc, nt * 512:(nt + 1) * 512], ps2,
                                    x_res[:, fc, nt * 512:(nt + 1) * 512],
                                    scale=gbs_fc2[:, fc:fc + 1],
                                    bias=gb_fc2[:, fc:fc + 1])
                            nc.sync.dma_start(out=out[fc * P:(fc + 1) * P, :],
                                              in_=x_res[:, fc, :])
    nc.compile()
    return nc


_CACHE = {}


def _runner(nc, n_cores=8):
    import jax
    import numpy as _np
    from jax.sharding import Mesh, PartitionSpec, NamedSharding
    from jax.experimental.shard_map import shard_map
    from concourse.bass2jax import _bass_exec_p, install_neuronx_cc_hook, partition_id_tensor

    install_neuronx_cc_hook()
    in_names, out_names, out_avals = [], [], []
    partition_name = nc.partition_id_tensor.name if nc.partition_id_tensor else None
    for alloc in nc.m.functions[0].allocations:
        if not isinstance(alloc, mybir.MemoryLocationSet):
            continue
        nm = alloc.memorylocations[0].name
        if alloc.kind == "ExternalInput":
            if nm != partition_name:
                in_names.append(nm)
        elif alloc.kind == "ExternalOutput":
            out_names.append(nm)
            out_avals.append(jax.core.ShapedArray(tuple(alloc.tensor_shape),
                                                  mybir.dt.np(alloc.dtype)))

    def _body(*args):
        operands = list(args)
        if partition_name is not None:
            operands.append(partition_id_tensor())
        outs = _bass_exec_p.bind(
            *operands,
            out_avals=tuple(out_avals),
            in_names=tuple(in_names + [partition_name] if partition_name else in_names),
            out_names=tuple(out_names),
            lowering_input_output_aliases=(),
            sim_require_finite=False,
            sim_require_nnan=False,
            nc=nc,
        )
        return tuple(outs)

    devices = jax.devices()[:n_cores]
    mesh = Mesh(_np.asarray(devices), ("core",))
    fn = jax.jit(shard_map(_body, mesh=mesh,
                           in_specs=(PartitionSpec("core"),) * len(in_names),
                           out_specs=(PartitionSpec("core"),) * len(out_names),
                           check_rep=False))

    def run(in_maps):
        concat = [_np.concatenate([_np.asarray(m[n]) for m in in_maps], axis=0)
                  for n in in_names]
        args = [jax.device_put(c, NamedSharding(mesh, PartitionSpec("core")))
                for c in concat]
        outs = fn(*args)
        jax.block_until_ready(outs)
        res = []
        for c in range(n_cores):
            d = {}
            for i, nm in enumerate(out_names):
                full = _np.asarray(outs[i])
                d[nm] = full.reshape(n_cores, *out_avals[i].shape)[c]
            res.append(d)
        return res

    return run


def kernel(**inputs):
    """Full (unsharded) inputs -> full (B, T, D) float32 output."""
    if "nc" not in _CACHE:
        _CACHE["nc"] = build_dit(n_cores=8)
        _CACHE["run"] = _runner(_CACHE["nc"], 8)
    in_maps = host_prep(**inputs)
    results = _CACHE["run"](in_maps)
    return host_post(results)


# revision 20
# speedup vs baseline: 2.0796x; 1.0554x over previous
"""nn_DiTBlock on 8 TRN2 NeuronCores: data-parallel over batch (B=8), one
batch element per core. Self-contained: builds the Bass/Tile kernel, shards
inputs on the host (transpose/pack/cast only), runs SPMD via bass2jax/PJRT,
gathers and un-transposes the output.

v2 design: fp8e4(e4m3)+DoubleRow matmuls for qkv/v/attn@v/proj/fc1/fc2
(weights host-prescaled x128, descale folded into evictions), bf16 for adaLN
and attention scores, f32 residual + PSUM. exp scaled by 1/32 (cancels in
softmax); k-rmsnorm folded into the exp's per-partition scale; odd heads'
attn@v written directly to PSUM partitions 64:128 (no partition-move DMAs);
single-pass MLP; multi-chunk batched weight DMAs."""

import numpy as np
from contextlib import ExitStack

import concourse.bass as bass
import concourse.mybir as mybir
import concourse.tile as tile
from concourse import bacc


F32 = mybir.dt.float32
F32R = mybir.dt.float32r
BF16 = mybir.dt.bfloat16
FP8 = mybir.dt.float8e4
AF = mybir.ActivationFunctionType
OP = mybir.AluOpType
DR = mybir.MatmulPerfMode.DoubleRow

B, T, D, H = 8, 1024, 1024, 16
HD = D // H          # 64
DM = 4 * D           # 4096
NCH = D // 128       # 8
MCH = DM // 128      # 32
P = 128
WS = 128.0           # fp8 weight pre-scale (host)
ISV = 1.0 / WS
ELN32 = -3.4657359027997265  # -ln(32): exp pre-scale so fp8 es stays < 240


def host_prep(x, c, g1, g2, gq, gk, Wqkv, bqkv, Wproj, bproj,
              Wfc1, bfc1, Wfc2, bfc2, Wada, bada):
    import ml_dtypes
    E4 = mybir.dt.np(FP8)

    def packT(W, npdt, scale=1.0):  # (F, K) -> (K//128, 128, F) contiguous
        Wt = np.ascontiguousarray(np.asarray(W, np.float32).T * scale).astype(npdt)
        K, F = Wt.shape
        return np.ascontiguousarray(Wt.reshape(K // 128, 128, F))

    f32 = np.float32
    com = {
        "wqkv": packT(Wqkv, E4, WS), "wproj": packT(Wproj, E4, WS),
        "wfc1": packT(Wfc1, E4, WS), "wfc2": packT(Wfc2, E4, WS),
        "wada": packT(Wada, ml_dtypes.bfloat16),
        "bqkv": np.asarray(bqkv, f32), "bproj": np.asarray(bproj, f32),
        "bfc1": np.asarray(bfc1, f32), "bfc2": np.asarray(bfc2, f32),
        "bada": np.asarray(bada, f32),
        "g": np.stack([np.asarray(g1)[0], np.asarray(g2)[0],
                       np.asarray(gq)[0], np.asarray(gk)[0]]).astype(f32),
    }
    in_maps = []
    for b in range(B):
        m = dict(com)
        m["xt"] = np.ascontiguousarray(np.asarray(x[b], f32).T)
        m["cvec"] = np.asarray(c[b], f32)
        in_maps.append(m)
    return in_maps


def host_post(results):
    return np.ascontiguousarray(
        np.stack([r["out"].T for r in results]).astype(np.float32))


def col_ap(handle, nch):
    """DRAM (nch*128,) viewed as [128, nch]: tile[p, ch] = v[ch*128+p]."""
    return bass.AP(tensor=handle, offset=0, ap=[[1, P], [P, nch]])


def bc_ap(handle, n, offset=0):
    """DRAM (n,) broadcast-read to [128, n] (partition stride 0)."""
    return bass.AP(tensor=handle, offset=offset, ap=[[0, P], [1, n]])


def wload_ap(handle, kch, cols, col0):
    """DRAM weight pack [KCH,128,F] -> [128, kch, cols] AP at col offset."""
    F = handle.shape[2]
    return bass.AP(tensor=handle, offset=col0,
                   ap=[[F, P], [P * F, kch], [1, cols]])


def build_dit(n_cores=8):
    nc = bacc.Bacc("TRN2", target_bir_lowering=False, debug=False,
                   num_devices=n_cores)

    xt = nc.dram_tensor("xt", [D, T], F32, kind="ExternalInput")
    cin = nc.dram_tensor("cvec", [D], F32, kind="ExternalInput")
    g = nc.dram_tensor("g", [4], F32, kind="ExternalInput")
    wqkv = nc.dram_tensor("wqkv", [NCH, P, 3 * D], FP8, kind="ExternalInput")
    wproj = nc.dram_tensor("wproj", [NCH, P, D], FP8, kind="ExternalInput")
    wfc1 = nc.dram_tensor("wfc1", [NCH, P, DM], FP8, kind="ExternalInput")
    wfc2 = nc.dram_tensor("wfc2", [MCH, P, D], FP8, kind="ExternalInput")
    wada = nc.dram_tensor("wada", [NCH, P, 6 * D], BF16, kind="ExternalInput")
    bqkv = nc.dram_tensor("bqkv", [3 * D], F32, kind="ExternalInput")
    bproj = nc.dram_tensor("bproj", [D], F32, kind="ExternalInput")
    bfc1 = nc.dram_tensor("bfc1", [DM], F32, kind="ExternalInput")
    bfc2 = nc.dram_tensor("bfc2", [D], F32, kind="ExternalInput")
    bada = nc.dram_tensor("bada", [6 * D], F32, kind="ExternalInput")
    out = nc.dram_tensor("out", [D, T], F32, kind="ExternalOutput")

    with tile.TileContext(nc, pool_alloc_mode="queue") as tc:
        with ExitStack() as X:
            const = X.enter_context(tc.tile_pool(name="const", bufs=1))
            resid = X.enter_context(tc.tile_pool(name="resid", bufs=1))
            dram = X.enter_context(tc.tile_pool(name="dram", bufs=1, space="DRAM"))

            # ---------------- constants ----------------
            g_bc = const.tile([P, 4], F32)
            nc.sync.dma_start(out=g_bc, in_=bc_ap(g, 4))
            gsq = const.tile([P, 4], F32)
            nc.vector.tensor_tensor(gsq, g_bc, g_bc, OP.mult)
            ginv2 = const.tile([P, 4], F32)
            nc.vector.reciprocal(ginv2, gsq)
            # Rsqrt scales: rinv = rsqrt(ps * scl)
            scl_n1 = const.tile([P, 1], F32)
            nc.vector.tensor_scalar_mul(scl_n1, ginv2[:, 0:1], 1.0 / D)
            scl_n2 = const.tile([P, 1], F32)
            nc.vector.tensor_scalar_mul(scl_n2, ginv2[:, 1:2], 1.0 / D)
            scl_q = const.tile([P, 1], F32)
            nc.vector.tensor_copy(scl_q, ginv2[:, 2:3])
            scl_k = const.tile([P, 1], F32)
            nc.vector.tensor_scalar_mul(scl_k, ginv2[:, 3:4], 1.0 / HD)

            ones1_f = const.tile([P, 1], F32)
            nc.gpsimd.memset(ones1_f, 1.0)
            ones1 = const.tile([P, 1], BF16)
            nc.vector.tensor_copy(ones1, ones1_f)
            onesh_f = const.tile([P, 2], F32)
            nc.gpsimd.memset(onesh_f, 0.0)
            nc.gpsimd.memset(onesh_f[0:64, 0:1], 1.0)
            nc.gpsimd.memset(onesh_f[64:128, 1:2], 1.0)
            onesh = const.tile([P, 2], BF16)
            nc.vector.tensor_copy(onesh, onesh_f)

            bqkv_c = const.tile([P, 3 * D // P], F32)
            nc.sync.dma_start(out=bqkv_c, in_=col_ap(bqkv, 3 * D // P))
            bproj_c = const.tile([P, NCH], F32)
            nc.sync.dma_start(out=bproj_c, in_=col_ap(bproj, NCH))
            bfc1_c = const.tile([P, MCH], F32)
            nc.sync.dma_start(out=bfc1_c, in_=col_ap(bfc1, MCH))
            bfc2_c = const.tile([P, NCH], F32)
            nc.sync.dma_start(out=bfc2_c, in_=col_ap(bfc2, NCH))
            vbias_bc = const.tile([P, D], F32)
            nc.sync.dma_start(out=vbias_bc, in_=bc_ap(bqkv, D, offset=2 * D))
            eln32_c = const.tile([P, 1], F32)
            nc.gpsimd.memset(eln32_c, ELN32)

            x_res = resid.tile([P, NCH, T], F32)
            for j in range(4):
                nc.sync.dma_start(
                    out=x_res[:, 2 * j:2 * j + 2, :],
                    in_=bass.AP(tensor=xt, offset=2 * j * P * T,
                                ap=[[T, P], [P * T, 2], [1, T]]))

            c_pm = const.tile([P, NCH], F32)
            nc.sync.dma_start(out=c_pm, in_=col_ap(cin, NCH))
            cs_pm = const.tile([P, NCH], BF16)
            nc.scalar.activation(cs_pm, c_pm, AF.Silu)

            # ---------------- adaLN (bf16) ----------------
            ada_scr = dram.tile([1, 6 * D], F32)
            ada_sb = const.tile([1, 6 * D], F32)
            with tc.tile_pool(name="wadap", bufs=3) as wp, \
                 tc.tile_pool(name="psA", bufs=2, space="PSUM") as psA:
                for nb in range(12):
                    wt = wp.tile([P, NCH, 512], BF16, name="wt")
                    nc.sync.dma_start(out=wt, in_=wload_ap(wada, NCH, 512, nb * 512))
                    pa = psA.tile([1, 512], F32, name="pa")
                    for d in range(NCH):
                        nc.tensor.matmul(pa, cs_pm[:, d:d + 1], wt[:, d, :],
                                         start=(d == 0), stop=(d == NCH - 1))
                    nc.vector.tensor_copy(ada_sb[:, nb * 512:(nb + 1) * 512], pa)
            nc.sync.dma_start(out=ada_scr, in_=ada_sb)
            adaT = const.tile([P, 48], F32)
            nc.sync.dma_start(out=adaT, in_=bass.AP(tensor=ada_scr.tensor, offset=0,
                                                    ap=[[1, P], [P, 48]]))
            badaT = const.tile([P, 48], F32)
            nc.sync.dma_start(out=badaT, in_=col_ap(bada, 48))
            nc.vector.tensor_tensor(adaT, adaT, badaT, OP.add)
            # cols: shift_msa 0:8 | scale_msa 8:16 | gate_msa 16:24
            #       shift_mlp 24:32 | scale_mlp 32:40 | gate_mlp 40:48
            nc.vector.tensor_scalar_add(adaT[:, 8:16], adaT[:, 8:16], 1.0)
            nc.vector.tensor_scalar_add(adaT[:, 32:40], adaT[:, 32:40], 1.0)
            gb_proj = const.tile([P, NCH], F32)
            nc.vector.tensor_tensor(gb_proj, adaT[:, 16:24], bproj_c, OP.mult)
            gbs_proj = const.tile([P, NCH], F32)
            nc.vector.tensor_scalar_mul(gbs_proj, adaT[:, 16:24], ISV)
            gb_fc2 = const.tile([P, NCH], F32)
            nc.vector.tensor_tensor(gb_fc2, adaT[:, 40:48], bfc2_c, OP.mult)
            gbs_fc2 = const.tile([P, NCH], F32)
            nc.vector.tensor_scalar_mul(gbs_fc2, adaT[:, 40:48], ISV)

            def norm_modulate(scl, sh_col, sc_col, h_out):
                """x_res (f32) -> h_out (fp8): rmsnorm + adaLN modulate."""
                with tc.tile_pool(name="sqp", bufs=3) as sqp, \
                     tc.tile_pool(name="psN", bufs=1, space="PSUM") as psN, \
                     tc.tile_pool(name="nrm", bufs=1) as nrm:
                    pss = psN.tile([1, T], F32, name="pss")
                    for j in range(NCH):
                        xsq = sqp.tile([P, T], BF16, name="xsq")
                        nc.scalar.activation(xsq, x_res[:, j, :], AF.Square)
                        for t2 in range(2):
                            nc.tensor.matmul(pss[:, t2 * 512:(t2 + 1) * 512],
                                             ones1, xsq[:, t2 * 512:(t2 + 1) * 512],
                                             start=(j == 0), stop=(j == NCH - 1))
                    rr = nrm.tile([1, T], F32, name="rr")
                    for t2 in range(2):
                        nc.scalar.activation(rr[:, t2 * 512:(t2 + 1) * 512],
                                             pss[:, t2 * 512:(t2 + 1) * 512],
                                             AF.Sqrt, scale=scl[0:1, :])
                    rinv = nrm.tile([1, T], F32, name="rinv")
                    nc.vector.reciprocal(rinv, rr)
                    rbc = nrm.tile([P, T], F32, name="rbc")
                    nc.gpsimd.partition_broadcast(rbc, rinv)
                    with tc.tile_pool(name="xnp", bufs=3) as xnp:
                        for j in range(NCH):
                            xn = xnp.tile([P, T], F32, name="xn")
                            nc.vector.tensor_tensor(xn, x_res[:, j, :], rbc, OP.mult)
                            nc.gpsimd.tensor_scalar(h_out[:, j, :], xn,
                                                    adaT[:, sc_col + j:sc_col + j + 1],
                                                    adaT[:, sh_col + j:sh_col + j + 1],
                                                    OP.mult, OP.add)

            att = ExitStack()
            h1p = att.enter_context(tc.tile_pool(name="h1p", bufs=1, side="right"))
            h1 = h1p.tile([P, NCH, T], FP8)
            # ------------ norm1 + modulate ------------
            norm_modulate(scl_n1, 0, 8, h1)

            # ------------ q, k (feature-major bf16) + per-head rmsnorm ------------
            qp_ = att.enter_context(tc.tile_pool(name="qp_", bufs=1))
            kp_ = att.enter_context(tc.tile_pool(name="kp_", bufs=1))
            q_t = qp_.tile([P, NCH, T], BF16)
            k_t = kp_.tile([P, NCH, T], BF16)
            rkcp = att.enter_context(tc.tile_pool(name="rkcp", bufs=1))
            rkc = rkcp.tile([P, H, NCH], F32)  # 1/|k| per k-token, head-major

            with tc.tile_pool(name="wqp", bufs=2) as wqp, \
                 tc.tile_pool(name="sqq", bufs=2) as sqq, \
                 tc.tile_pool(name="psD", bufs=2, space="PSUM") as psD, \
                 tc.tile_pool(name="psR", bufs=1, space="PSUM") as psR, \
                 tc.tile_pool(name="nrq", bufs=2) as nrq:
                for fc in range(16):  # q: 0..7, k: 8..15
                    if fc % 4 == 0:
                        wt = wqp.tile([P, NCH, 512], FP8, name="wt")
                        nc.sync.dma_start(out=wt, in_=wload_ap(wqkv, NCH, 512, fc * P))
                    tgt = q_t if fc < 8 else k_t
                    ch = fc % 8
                    ps = [psD.tile([P, 512], F32, name="ps") for _ in range(2)]
                    for nt in range(2):
                        for dp in range(4):
                            nc.tensor.matmul(
                                ps[nt],
                                wt[:, 2 * dp:2 * dp + 2, (fc % 4) * P:(fc % 4 + 1) * P],
                                h1[:, 2 * dp:2 * dp + 2, nt * 512:(nt + 1) * 512],
                                start=(dp == 0), stop=(dp == 3), perf_mode=DR)
                        # evict: (psum/128 + bias) -> bf16 on Act (idle here)
                        nc.scalar.activation(tgt[:, ch, nt * 512:(nt + 1) * 512],
                                             ps[nt], AF.Identity, scale=ISV,
                                             bias=bqkv_c[:, fc:fc + 1])
                    # sum of squares per head
                    sq = sqq.tile([P, T], BF16, name="sq")
                    nc.vector.tensor_tensor(sq, tgt[:, ch, :], tgt[:, ch, :], OP.mult)
                    if fc < 8:
                        # q: per-half sums, each in its own row-0 psum tile;
                        # broadcasts always source partition 0 into full tiles
                        for hfq in range(2):
                            prh = psR.tile([1, T], F32, name=f"prh{hfq}")
                            for nt in range(2):
                                nc.tensor.matmul(
                                    prh[:, nt * 512:(nt + 1) * 512],
                                    onesh[:, hfq:hfq + 1],
                                    sq[:, nt * 512:(nt + 1) * 512],
                                    start=True, stop=True)
                            rr2 = nrq.tile([1, T], BF16, name=f"rr2{hfq}")
                            nc.scalar.activation(rr2, prh, AF.Sqrt,
                                                 scale=scl_q[0:1, :])
                            rinv_sb = nrq.tile([1, T], BF16, name=f"ri{hfq}")
                            with nc.allow_low_precision(reason="1/|q| bf16"):
                                nc.vector.reciprocal(rinv_sb, rr2)
                            rbcq = nrq.tile([P, T], BF16, name=f"rbcq{hfq}")
                            nc.gpsimd.partition_broadcast(rbcq, rinv_sb)
                            hs = slice(64 * hfq, 64 * (hfq + 1))
                            nc.vector.tensor_tensor(q_t[hs, ch, :], q_t[hs, ch, :],
                                                    rbcq[hs, :], OP.mult)
                    else:
                        # k: sums token-major [128, NCH] per head -> 1/|k| into
                        # rkc, consumed as the exp's per-partition scale.
                        for j in range(2):
                            hidx = 2 * (fc - 8) + j
                            pkn = psR.tile([P, NCH], F32, name="pkn")
                            for kt in range(NCH):
                                nc.tensor.matmul(
                                    pkn[:, kt:kt + 1],
                                    sq[64 * j:64 * (j + 1), kt * P:(kt + 1) * P],
                                    ones1[64 * j:64 * (j + 1), :],
                                    start=True, stop=True)
                            rrk = nrq.tile([P, NCH], F32, name="rrk")
                            nc.scalar.activation(rrk, pkn, AF.Sqrt, scale=scl_k)
                            nc.vector.reciprocal(rkc[:, hidx, :], rrk)

            # ------------ v (token-major fp8, ones-augmented) ------------
            # vx per-head 128-col slot: even h = [v(0:64) | ones@64 | 0],
            # odd h = [0 | ones@63 | v(64:128)]; attn@v DR outputs are then
            # always full [128, N] (walrus requires that) and odd heads land
            # on PSUM partitions 64:128 directly.
            vxp = att.enter_context(tc.tile_pool(name="vxp", bufs=1))
            vx = vxp.tile([P, NCH, H, P], FP8)   # [ktok][ktc][head][col]
            nc.gpsimd.memset(vx, 0.0)
            for h in range(H):
                oc = HD if h % 2 == 0 else 0
                nc.gpsimd.memset(vx[:, :, h, oc:oc + 1], 1.0)
            with tc.tile_pool(name="wvp", bufs=2) as wvp, \
                 tc.tile_pool(name="psV", bufs=3, space="PSUM") as psV:
                for nq in range(2):
                    wv = wvp.tile([P, NCH, 512], FP8, name="wv")
                    nc.sync.dma_start(out=wv,
                                      in_=wload_ap(wqkv, NCH, 512, 2 * D + nq * 512))
                    for t8 in range(NCH):
                        pv = psV.tile([P, 512], F32, name="pv")
                        for dp in range(4):
                            nc.tensor.matmul(
                                pv, h1[:, 2 * dp:2 * dp + 2, t8 * P:(t8 + 1) * P],
                                wv[:, 2 * dp:2 * dp + 2, :],
                                start=(dp == 0), stop=(dp == 3), perf_mode=DR)
                        # heads alternate col-base 0 (even) / 64 (odd) in vx
                        vblk = vx[:, t8, :, :].rearrange(
                            "p h c -> p (h c)").rearrange(
                            "p (i r) -> p i r", r=256)  # [P, 8, 256]
                        for par in range(2):
                            nc.vector.scalar_tensor_tensor(
                                vblk[:, 4 * nq:4 * nq + 4,
                                     192 * par:192 * par + HD],
                                pv.rearrange("p (i r) -> p i r", r=128)[
                                    :, :, par * HD:(par + 1) * HD], ISV,
                                vbias_bc[:, nq * 512:(nq + 1) * 512].rearrange(
                                    "p (i r) -> p i r", r=128)[
                                    :, :, par * HD:(par + 1) * HD],
                                OP.mult, OP.add)

            # ------------ attention ------------
            oTp = att.enter_context(tc.tile_pool(name="oTp", bufs=1, side="right"))
            oT = oTp.tile([P, NCH, T], FP8)
            with tc.tile_pool(name="esp", bufs=2) as esp, \
                 tc.tile_pool(name="psS", bufs=2, space="PSUM") as psS, \
                 tc.tile_pool(name="psO", bufs=2, space="PSUM") as psO, \
                 tc.tile_pool(name="onp", bufs=4) as onp:
                for h in range(H):
                    hc, hf = h // 2, h % 2
                    rq = slice(64 * hf, 64 * (hf + 1))
                    es_h = esp.tile([P, NCH, T], FP8, name="es")
                    for ktc in range(NCH):
                        psc = psS.tile([P, T], F32, name="psc")
                        for qt in range(2):
                            nc.tensor.matmul(psc[:, qt * 512:(qt + 1) * 512],
                                             k_t[rq, hc, ktc * P:(ktc + 1) * P],
                                             q_t[rq, hc, qt * 512:(qt + 1) * 512],
                                             start=True, stop=True)
                        nc.scalar.activation(es_h[:, ktc, :], psc, AF.Exp,
                                             bias=eln32_c, scale=rkc[:, h, ktc:ktc + 1])
                    for qt in range(2):
                        qs = slice(qt * 512, (qt + 1) * 512)
                        po = psO.tile([P, 512], F32, name="po")
                        rs = onp.tile([P, 512], F32, name="rs")
                        rsb = onp.tile([P, 512], F32, name="rsb")
                        for kp in range(4):
                            nc.tensor.matmul(
                                po, vx[:, 2 * kp:2 * kp + 2, h, :],
                                es_h[:, 2 * kp:2 * kp + 2, qs],
                                start=(kp == 0), stop=(kp == 3), perf_mode=DR)
                        if hf == 0:
                            # denom at row 64: recip there, DMA row to
                            # partition 0, broadcast full, use rows 0:64
                            nc.vector.reciprocal(rs[64:65, :], po[64:65, :])
                            rse = onp.tile([1, 512], F32, name="rse")
                            nc.sync.dma_start(out=rse, in_=rs[64:65, :])
                            nc.gpsimd.partition_broadcast(rsb, rse)
                            nc.vector.tensor_tensor(oT[0:64, hc, qs], po[0:64, :],
                                                    rsb[0:64, :], OP.mult)
                        else:
                            # denom at row 0: broadcast full, use rows 64:128
                            nc.vector.reciprocal(rs[0:1, :], po[0:1, :])
                            nc.gpsimd.partition_broadcast(rsb, rs[0:1, :])
                            nc.vector.tensor_tensor(oT[64:128, hc, qs], po[64:128, :],
                                                    rsb[64:128, :], OP.mult)

            # ------------ proj + residual ------------
            with tc.tile_pool(name="wpp", bufs=2) as wpp, \
                 tc.tile_pool(name="psP", bufs=3, space="PSUM") as psP:
                for half in range(2):
                    wpj = wpp.tile([P, NCH, 512], FP8, name="wpj")
                    nc.sync.dma_start(out=wpj, in_=wload_ap(wproj, NCH, 512, half * 512))
                    for fi in range(4):
                        fc = half * 4 + fi
                        for nt in range(2):
                            pp = psP.tile([P, 512], F32, name="pp")
                            for dp in range(4):
                                nc.tensor.matmul(
                                    pp, wpj[:, 2 * dp:2 * dp + 2, fi * P:(fi + 1) * P],
                                    oT[:, 2 * dp:2 * dp + 2, nt * 512:(nt + 1) * 512],
                                    start=(dp == 0), stop=(dp == 3), perf_mode=DR)
                            nc.vector.affine_then_add(
                                x_res[:, fc, nt * 512:(nt + 1) * 512], pp,
                                x_res[:, fc, nt * 512:(nt + 1) * 512],
                                scale=gbs_proj[:, fc:fc + 1],
                                bias=gb_proj[:, fc:fc + 1])

            att.close()  # free h1, q/k, vx, oT, rkc

            # ------------ norm2 + modulate + MLP (single pass, fp8) ------------
            with tc.tile_pool(name="h2p", bufs=1) as h2p, \
                 tc.tile_pool(name="gactp", bufs=1, side="right") as gactp:
                h2 = h2p.tile([P, NCH, T], FP8)
                norm_modulate(scl_n2, 24, 32, h2)
                gact = gactp.tile([P, MCH, T], FP8)
                with tc.tile_pool(name="wf1p", bufs=2) as wf1p, \
                     tc.tile_pool(name="psM", bufs=2, space="PSUM") as psM:
                    for mg in range(8):
                        w1 = wf1p.tile([P, NCH, 512], FP8, name="w1")
                        nc.sync.dma_start(out=w1, in_=wload_ap(wfc1, NCH, 512, mg * 512))
                        for mi in range(4):
                            m = mg * 4 + mi
                            psm = psM.tile([P, T], F32, name="psm")
                            for nt in range(2):
                                for dp in range(4):
                                    nc.tensor.matmul(
                                        psm[:, nt * 512:(nt + 1) * 512],
                                        w1[:, 2 * dp:2 * dp + 2, mi * P:(mi + 1) * P],
                                        h2[:, 2 * dp:2 * dp + 2, nt * 512:(nt + 1) * 512],
                                        start=(dp == 0), stop=(dp == 3), perf_mode=DR)
                            nc.scalar.activation(gact[:, m, :], psm,
                                                 AF.Gelu_apprx_tanh, scale=ISV,
                                                 bias=bfc1_c[:, m:m + 1])
                with tc.tile_pool(name="wf2p", bufs=2) as wf2p, \
                     tc.tile_pool(name="psM2", bufs=3, space="PSUM") as psM2:
                    for half in range(2):
                        w2 = wf2p.tile([P, MCH, 512], FP8, name="w2")
                        nc.sync.dma_start(out=w2, in_=wload_ap(wfc2, MCH, 512, half * 512))
                        for fi in range(4):
                            fc = half * 4 + fi
                            for nt in range(2):
                                ps2 = psM2.tile([P, 512], F32, name="ps2")
                                for dp in range(16):
                                    nc.tensor.matmul(
                                        ps2,
                                        w2[:, 2 * dp:2 * dp + 2, fi * P:(fi + 1) * P],
                                        gact[:, 2 * dp:2 * dp + 2,
                                             nt * 512:(nt + 1) * 512],
                                        start=(dp == 0), stop=(dp == 15), perf_mode=DR)
                                nc.vector.affine_then_add(
                                    x_res[:, fc, nt * 512:(nt + 1) * 512], ps2,
                                    x_res[:, fc, nt * 512:(nt + 1) * 512],
                                    scale=gbs_fc2[:, fc:fc + 1],
                                    bias=gb_fc2[:, fc:fc + 1])
                            nc.sync.dma_start(out=out[fc * P:(fc + 1) * P, :],
                                              in_=x_res[:, fc, :])
    nc.compile()
    return nc


_CACHE = {}


def _runner(nc, n_cores=8):
    import jax
    import numpy as _np
    from jax.sharding import Mesh, PartitionSpec, NamedSharding
    from jax.experimental.shard_map import shard_map
    from concourse.bass2jax import _bass_exec_p, install_neuronx_cc_hook, partition_id_tensor

    install_neuronx_cc_hook()
    in_names, out_names, out_avals = [], [], []
    partition_name = nc.partition_id_tensor.name if nc.partition_id_tensor else None
    for alloc in nc.m.functions[0].allocations:
        if not isinstance(alloc, mybir.MemoryLocationSet):
            continue
        nm = alloc.memorylocations[0].name
        if alloc.kind == "ExternalInput":
            if nm != partition_name:
                in_names.append(nm)
        elif alloc.kind == "ExternalOutput":
            out_names.append(nm)
            out_avals.append(jax.core.ShapedArray(tuple(alloc.tensor_shape),
                                                  mybir.dt.np(alloc.dtype)))

    def _body(*args):
        operands = list(args)
        if partition_name is not None:
            operands.append(partition_id_tensor())
        outs = _bass_exec_p.bind(
            *operands,
            out_avals=tuple(out_avals),
            in_names=tuple(in_names + [partition_name] if partition_name else in_names),
            out_names=tuple(out_names),
            lowering_input_output_aliases=(),
            sim_require_finite=False,
            sim_require_nnan=False,
            nc=nc,
        )
        return tuple(outs)

    devices = jax.devices()[:n_cores]
    mesh = Mesh(_np.asarray(devices), ("core",))
    fn = jax.jit(shard_map(_body, mesh=mesh,
                           in_specs=(PartitionSpec("core"),) * len(in_names),
                           out_specs=(PartitionSpec("core"),) * len(out_names),
                           check_rep=False))

    def run(in_maps):
        concat = [_np.concatenate([_np.asarray(m[n]) for m in in_maps], axis=0)
                  for n in in_names]
        args = [jax.device_put(c, NamedSharding(mesh, PartitionSpec("core")))
                for c in concat]
        outs = fn(*args)
        jax.block_until_ready(outs)
        res = []
        for c in range(n_cores):
            d = {}
            for i, nm in enumerate(out_names):
                full = _np.asarray(outs[i])
                d[nm] = full.reshape(n_cores, *out_avals[i].shape)[c]
            res.append(d)
        return res

    return run


def kernel(**inputs):
    """Full (unsharded) inputs -> full (B, T, D) float32 output."""
    if "nc" not in _CACHE:
        _CACHE["nc"] = build_dit(n_cores=8)
        _CACHE["run"] = _runner(_CACHE["nc"], 8)
    in_maps = host_prep(**inputs)
    results = _CACHE["run"](in_maps)
    return host_post(results)


# revision 29
# speedup vs baseline: 2.1030x; 1.0112x over previous
"""nn_DiTBlock on 8 TRN2 NeuronCores: data-parallel over batch (B=8), one
batch element per core. Self-contained: builds the Bass/Tile kernel, shards
inputs on the host (transpose/pack/cast only), runs SPMD via bass2jax/PJRT,
gathers and un-transposes the output.

v2 design: fp8e4(e4m3)+DoubleRow matmuls for qkv/v/attn@v/proj/fc1/fc2
(weights host-prescaled x128, descale folded into evictions), bf16 for adaLN
and attention scores, f32 residual + PSUM. exp scaled by 1/32 (cancels in
softmax); k-rmsnorm folded into the exp's per-partition scale; odd heads'
attn@v written directly to PSUM partitions 64:128 (no partition-move DMAs);
single-pass MLP; multi-chunk batched weight DMAs."""

import numpy as np
from contextlib import ExitStack

import concourse.bass as bass
import concourse.mybir as mybir
import concourse.tile as tile
from concourse import bacc


F32 = mybir.dt.float32
F32R = mybir.dt.float32r
BF16 = mybir.dt.bfloat16
FP8 = mybir.dt.float8e4
AF = mybir.ActivationFunctionType
OP = mybir.AluOpType
DR = mybir.MatmulPerfMode.DoubleRow

B, T, D, H = 8, 1024, 1024, 16
HD = D // H          # 64
DM = 4 * D           # 4096
NCH = D // 128       # 8
MCH = DM // 128      # 32
P = 128
WS = 128.0           # fp8 weight pre-scale (host)
ISV = 1.0 / WS
ELN32 = -3.4657359027997265  # -ln(32): exp pre-scale so fp8 es stays < 240


def host_prep(x, c, g1, g2, gq, gk, Wqkv, bqkv, Wproj, bproj,
              Wfc1, bfc1, Wfc2, bfc2, Wada, bada):
    import ml_dtypes
    E4 = mybir.dt.np(FP8)

    def packT(W, npdt, scale=1.0):  # (F, K) -> (K//128, 128, F) contiguous
        Wt = np.ascontiguousarray(np.asarray(W, np.float32).T * scale).astype(npdt)
        K, F = Wt.shape
        return np.ascontiguousarray(Wt.reshape(K // 128, 128, F))

    f32 = np.float32
    com = {
        "wqkv": packT(Wqkv, E4, WS), "wproj": packT(Wproj, E4, WS),
        "wfc1": packT(Wfc1, E4, WS), "wfc2": packT(Wfc2, E4, WS),
        "wada": packT(Wada, ml_dtypes.bfloat16),
        "bqkv": np.asarray(bqkv, f32), "bproj": np.asarray(bproj, f32),
        "bfc1": np.asarray(bfc1, f32), "bfc2": np.asarray(bfc2, f32),
        "bada": np.asarray(bada, f32),
        "g": np.stack([np.asarray(g1)[0], np.asarray(g2)[0],
                       np.asarray(gq)[0], np.asarray(gk)[0]]).astype(f32),
    }
    in_maps = []
    for b in range(B):
        m = dict(com)
        m["xt"] = np.ascontiguousarray(np.asarray(x[b], f32).T)
        m["cvec"] = np.asarray(c[b], f32)
        in_maps.append(m)
    return in_maps


def host_post(results):
    return np.ascontiguousarray(
        np.stack([r["out"].T for r in results]).astype(np.float32))


def col_ap(handle, nch):
    """DRAM (nch*128,) viewed as [128, nch]: tile[p, ch] = v[ch*128+p]."""
    return bass.AP(tensor=handle, offset=0, ap=[[1, P], [P, nch]])


def bc_ap(handle, n, offset=0):
    """DRAM (n,) broadcast-read to [128, n] (partition stride 0)."""
    return bass.AP(tensor=handle, offset=offset, ap=[[0, P], [1, n]])


def wload_ap(handle, kch, cols, col0):
    """DRAM weight pack [KCH,128,F] -> [128, kch, cols] AP at col offset."""
    F = handle.shape[2]
    return bass.AP(tensor=handle, offset=col0,
                   ap=[[F, P], [P * F, kch], [1, cols]])


def build_dit(n_cores=8):
    nc = bacc.Bacc("TRN2", target_bir_lowering=False, debug=False,
                   num_devices=n_cores)

    xt = nc.dram_tensor("xt", [D, T], F32, kind="ExternalInput")
    cin = nc.dram_tensor("cvec", [D], F32, kind="ExternalInput")
    g = nc.dram_tensor("g", [4], F32, kind="ExternalInput")
    wqkv = nc.dram_tensor("wqkv", [NCH, P, 3 * D], FP8, kind="ExternalInput")
    wproj = nc.dram_tensor("wproj", [NCH, P, D], FP8, kind="ExternalInput")
    wfc1 = nc.dram_tensor("wfc1", [NCH, P, DM], FP8, kind="ExternalInput")
    wfc2 = nc.dram_tensor("wfc2", [MCH, P, D], FP8, kind="ExternalInput")
    wada = nc.dram_tensor("wada", [NCH, P, 6 * D], BF16, kind="ExternalInput")
    bqkv = nc.dram_tensor("bqkv", [3 * D], F32, kind="ExternalInput")
    bproj = nc.dram_tensor("bproj", [D], F32, kind="ExternalInput")
    bfc1 = nc.dram_tensor("bfc1", [DM], F32, kind="ExternalInput")
    bfc2 = nc.dram_tensor("bfc2", [D], F32, kind="ExternalInput")
    bada = nc.dram_tensor("bada", [6 * D], F32, kind="ExternalInput")
    out = nc.dram_tensor("out", [D, T], F32, kind="ExternalOutput")

    with tile.TileContext(nc, pool_alloc_mode="queue") as tc:
        with ExitStack() as X:
            const = X.enter_context(tc.tile_pool(name="const", bufs=1))
            resid = X.enter_context(tc.tile_pool(name="resid", bufs=1))
            dram = X.enter_context(tc.tile_pool(name="dram", bufs=1, space="DRAM"))

            # ---------------- constants ----------------
            g_bc = const.tile([P, 4], F32)
            nc.sync.dma_start(out=g_bc, in_=bc_ap(g, 4))
            gsq = const.tile([P, 4], F32)
            nc.vector.tensor_tensor(gsq, g_bc, g_bc, OP.mult)
            ginv2 = const.tile([P, 4], F32)
            nc.vector.reciprocal(ginv2, gsq)
            # Rsqrt scales: rinv = rsqrt(ps * scl)
            scl_n1 = const.tile([P, 1], F32)
            nc.vector.tensor_scalar_mul(scl_n1, ginv2[:, 0:1], 1.0 / D)
            scl_n2 = const.tile([P, 1], F32)
            nc.vector.tensor_scalar_mul(scl_n2, ginv2[:, 1:2], 1.0 / D)
            scl_q = const.tile([P, 1], F32)
            nc.vector.tensor_copy(scl_q, ginv2[:, 2:3])
            scl_k = const.tile([P, 1], F32)
            nc.vector.tensor_scalar_mul(scl_k, ginv2[:, 3:4], 1.0 / HD)

            ones1_f = const.tile([P, 1], F32)
            nc.gpsimd.memset(ones1_f, 1.0)
            ones1 = const.tile([P, 1], BF16)
            nc.vector.tensor_copy(ones1, ones1_f)
            onesh_f = const.tile([P, 2], F32)
            nc.gpsimd.memset(onesh_f, 0.0)
            nc.gpsimd.memset(onesh_f[0:64, 0:1], 1.0)
            nc.gpsimd.memset(onesh_f[64:128, 1:2], 1.0)
            onesh = const.tile([P, 2], BF16)
            nc.vector.tensor_copy(onesh, onesh_f)

            bqkv_c = const.tile([P, 3 * D // P], F32)
            nc.sync.dma_start(out=bqkv_c, in_=col_ap(bqkv, 3 * D // P))
            bproj_c = const.tile([P, NCH], F32)
            nc.sync.dma_start(out=bproj_c, in_=col_ap(bproj, NCH))
            bfc1_c = const.tile([P, MCH], F32)
            nc.sync.dma_start(out=bfc1_c, in_=col_ap(bfc1, MCH))
            bfc2_c = const.tile([P, NCH], F32)
            nc.sync.dma_start(out=bfc2_c, in_=col_ap(bfc2, NCH))
            vbias_bc = const.tile([P, D], F32)
            nc.sync.dma_start(out=vbias_bc, in_=bc_ap(bqkv, D, offset=2 * D))
            eln32_c = const.tile([P, 1], F32)
            nc.gpsimd.memset(eln32_c, ELN32)

            x_res = resid.tile([P, NCH, T], F32)
            for j in range(4):
                nc.sync.dma_start(
                    out=x_res[:, 2 * j:2 * j + 2, :],
                    in_=bass.AP(tensor=xt, offset=2 * j * P * T,
                                ap=[[T, P], [P * T, 2], [1, T]]))

            c_pm = const.tile([P, NCH], F32)
            nc.sync.dma_start(out=c_pm, in_=col_ap(cin, NCH))
            cs_pm = const.tile([P, NCH], BF16)
            nc.scalar.activation(cs_pm, c_pm, AF.Silu)

            # ---------------- adaLN (bf16) ----------------
            ada_scr = dram.tile([1, 6 * D], F32)
            ada_sb = const.tile([1, 6 * D], F32)
            with tc.tile_pool(name="wadap", bufs=3) as wp, \
                 tc.tile_pool(name="psA", bufs=2, space="PSUM") as psA:
                for nb in range(12):
                    wt = wp.tile([P, NCH, 512], BF16, name="wt")
                    nc.sync.dma_start(out=wt, in_=wload_ap(wada, NCH, 512, nb * 512))
                    pa = psA.tile([1, 512], F32, name="pa")
                    for d in range(NCH):
                        nc.tensor.matmul(pa, cs_pm[:, d:d + 1], wt[:, d, :],
                                         start=(d == 0), stop=(d == NCH - 1))
                    nc.vector.tensor_copy(ada_sb[:, nb * 512:(nb + 1) * 512], pa)
            nc.sync.dma_start(out=ada_scr, in_=ada_sb)
            adaT = const.tile([P, 48], F32)
            nc.sync.dma_start(out=adaT, in_=bass.AP(tensor=ada_scr.tensor, offset=0,
                                                    ap=[[1, P], [P, 48]]))
            badaT = const.tile([P, 48], F32)
            nc.sync.dma_start(out=badaT, in_=col_ap(bada, 48))
            nc.vector.tensor_tensor(adaT, adaT, badaT, OP.add)
            # cols: shift_msa 0:8 | scale_msa 8:16 | gate_msa 16:24
            #       shift_mlp 24:32 | scale_mlp 32:40 | gate_mlp 40:48
            nc.vector.tensor_scalar_add(adaT[:, 8:16], adaT[:, 8:16], 1.0)
            nc.vector.tensor_scalar_add(adaT[:, 32:40], adaT[:, 32:40], 1.0)
            gb_proj = const.tile([P, NCH], F32)
            nc.vector.tensor_tensor(gb_proj, adaT[:, 16:24], bproj_c, OP.mult)
            gbs_proj = const.tile([P, NCH], F32)
            nc.vector.tensor_scalar_mul(gbs_proj, adaT[:, 16:24], ISV)
            gb_fc2 = const.tile([P, NCH], F32)
            nc.vector.tensor_tensor(gb_fc2, adaT[:, 40:48], bfc2_c, OP.mult)
            gbs_fc2 = const.tile([P, NCH], F32)
            nc.vector.tensor_scalar_mul(gbs_fc2, adaT[:, 40:48], ISV)

            def norm_modulate(scl, sh_col, sc_col, h_out):
                """x_res (f32) -> h_out (fp8): rmsnorm + adaLN modulate.
                Token-halved so the consumer can start on half 0 while the
                producer of x_res is still finishing half 1."""
                with tc.tile_pool(name="sqp", bufs=3) as sqp, \
                     tc.tile_pool(name="psN", bufs=1, space="PSUM") as psN, \
                     tc.tile_pool(name="nrm", bufs=2) as nrm, \
                     tc.tile_pool(name="xnp", bufs=3) as xnp:
                    pss = psN.tile([1, T], F32, name="pss")
                    for t2 in range(2):
                        ts_ = slice(t2 * 512, (t2 + 1) * 512)
                        for j in range(NCH):
                            xsq = sqp.tile([P, 512], BF16, name="xsq")
                            nc.scalar.activation(xsq, x_res[:, j, ts_], AF.Square)
                            nc.tensor.matmul(pss[:, ts_], ones1, xsq,
                                             start=(j == 0), stop=(j == NCH - 1))
                        rr = nrm.tile([1, 512], F32, name="rr")
                        nc.scalar.activation(rr, pss[:, ts_], AF.Sqrt,
                                             scale=scl[0:1, :])
                        rinv = nrm.tile([1, 512], F32, name="rinv")
                        nc.vector.reciprocal(rinv, rr)
                        rbc = nrm.tile([P, 512], F32, name="rbc")
                        nc.gpsimd.partition_broadcast(rbc, rinv)
                        for j in range(NCH):
                            xn = xnp.tile([P, 512], F32, name="xn")
                            nc.vector.tensor_tensor(xn, x_res[:, j, ts_], rbc,
                                                    OP.mult)
                            nc.gpsimd.tensor_scalar(h_out[:, j, ts_], xn,
                                                    adaT[:, sc_col + j:sc_col + j + 1],
                                                    adaT[:, sh_col + j:sh_col + j + 1],
                                                    OP.mult, OP.add)

            att = ExitStack()
            h1p = att.enter_context(tc.tile_pool(name="h1p", bufs=1, side="right"))
            h1 = h1p.tile([P, NCH, T], FP8)
            # ------------ norm1 + modulate ------------
            norm_modulate(scl_n1, 0, 8, h1)

            # ------------ q, k (feature-major bf16) + per-head rmsnorm ------------
            qp_ = att.enter_context(tc.tile_pool(name="qp_", bufs=1))
            kp_ = att.enter_context(tc.tile_pool(name="kp_", bufs=1))
            q_t = qp_.tile([P, NCH, T], BF16)
            k_t = kp_.tile([P, NCH, T], BF16)
            rkcp = att.enter_context(tc.tile_pool(name="rkcp", bufs=1))
            rkc = rkcp.tile([P, H, NCH], F32)  # 1/|k| per k-token, head-major

            with tc.tile_pool(name="wqp", bufs=2) as wqp, \
                 tc.tile_pool(name="sqq", bufs=2) as sqq, \
                 tc.tile_pool(name="psD", bufs=2, space="PSUM") as psD, \
                 tc.tile_pool(name="psR", bufs=1, space="PSUM") as psR, \
                 tc.tile_pool(name="nrq", bufs=2) as nrq:
                for fc in range(16):  # q: 0..7, k: 8..15
                    if fc % 4 == 0:
                        wt = wqp.tile([P, NCH, 512], FP8, name="wt")
                        nc.sync.dma_start(out=wt, in_=wload_ap(wqkv, NCH, 512, fc * P))
                    tgt = q_t if fc < 8 else k_t
                    ch = fc % 8
                    ps = [psD.tile([P, 512], F32, name="ps") for _ in range(2)]
                    for nt in range(2):
                        for dp in range(4):
                            nc.tensor.matmul(
                                ps[nt],
                                wt[:, 2 * dp:2 * dp + 2, (fc % 4) * P:(fc % 4 + 1) * P],
                                h1[:, 2 * dp:2 * dp + 2, nt * 512:(nt + 1) * 512],
                                start=(dp == 0), stop=(dp == 3), perf_mode=DR)
                        # evict: (psum/128 + bias) -> bf16 on Act (idle here)
                        nc.scalar.activation(tgt[:, ch, nt * 512:(nt + 1) * 512],
                                             ps[nt], AF.Identity, scale=ISV,
                                             bias=bqkv_c[:, fc:fc + 1])
                    # sum of squares per head
                    sq = sqq.tile([P, T], BF16, name="sq")
                    nc.vector.tensor_tensor(sq, tgt[:, ch, :], tgt[:, ch, :], OP.mult)
                    if fc < 8:
                        # q: per-half sums, each in its own row-0 psum tile;
                        # broadcasts always source partition 0 into full tiles
                        for hfq in range(2):
                            prh = psR.tile([1, T], F32, name=f"prh{hfq}")
                            for nt in range(2):
                                nc.tensor.matmul(
                                    prh[:, nt * 512:(nt + 1) * 512],
                                    onesh[:, hfq:hfq + 1],
                                    sq[:, nt * 512:(nt + 1) * 512],
                                    start=True, stop=True)
                            rr2 = nrq.tile([1, T], BF16, name=f"rr2{hfq}")
                            nc.scalar.activation(rr2, prh, AF.Sqrt,
                                                 scale=scl_q[0:1, :])
                            rinv_sb = nrq.tile([1, T], BF16, name=f"ri{hfq}")
                            with nc.allow_low_precision(reason="1/|q| bf16"):
                                nc.vector.reciprocal(rinv_sb, rr2)
                            rbcq = nrq.tile([P, T], BF16, name=f"rbcq{hfq}")
                            nc.gpsimd.partition_broadcast(rbcq, rinv_sb)
                            hs = slice(64 * hfq, 64 * (hfq + 1))
                            nc.vector.tensor_tensor(q_t[hs, ch, :], q_t[hs, ch, :],
                                                    rbcq[hs, :], OP.mult)
                    else:
                        # k: sums token-major [128, NCH] per head -> 1/|k| into
                        # rkc, consumed as the exp's per-partition scale.
                        for j in range(2):
                            hidx = 2 * (fc - 8) + j
                            pkn = psR.tile([P, NCH], F32, name="pkn")
                            for kt in range(NCH):
                                nc.tensor.matmul(
                                    pkn[:, kt:kt + 1],
                                    sq[64 * j:64 * (j + 1), kt * P:(kt + 1) * P],
                                    ones1[64 * j:64 * (j + 1), :],
                                    start=True, stop=True)
                            rrk = nrq.tile([P, NCH], F32, name="rrk")
                            nc.scalar.activation(rrk, pkn, AF.Sqrt, scale=scl_k)
                            nc.vector.reciprocal(rkc[:, hidx, :], rrk)

            # ------------ v (token-major fp8, ones-augmented) ------------
            # vx per-head 128-col slot: even h = [v(0:64) | ones@64 | 0],
            # odd h = [0 | ones@63 | v(64:128)]; attn@v DR outputs are then
            # always full [128, N] (walrus requires that) and odd heads land
            # on PSUM partitions 64:128 directly.
            vxp = att.enter_context(tc.tile_pool(name="vxp", bufs=1))
            vx = vxp.tile([P, NCH, H, P], FP8)   # [ktok][ktc][head][col]
            nc.gpsimd.memset(vx, 0.0)
            for h in range(H):
                oc = HD if h % 2 == 0 else 0
                nc.gpsimd.memset(vx[:, :, h, oc:oc + 1], 1.0)
            with tc.tile_pool(name="wvp", bufs=2) as wvp, \
                 tc.tile_pool(name="psV", bufs=3, space="PSUM") as psV:
                for nq in range(2):
                    wv = wvp.tile([P, NCH, 512], FP8, name="wv")
                    nc.sync.dma_start(out=wv,
                                      in_=wload_ap(wqkv, NCH, 512, 2 * D + nq * 512))
                    for t8 in range(NCH):
                        pv = psV.tile([P, 512], F32, name="pv")
                        for dp in range(4):
                            nc.tensor.matmul(
                                pv, h1[:, 2 * dp:2 * dp + 2, t8 * P:(t8 + 1) * P],
                                wv[:, 2 * dp:2 * dp + 2, :],
                                start=(dp == 0), stop=(dp == 3), perf_mode=DR)
                        # heads alternate col-base 0 (even) / 64 (odd) in vx
                        vblk = vx[:, t8, :, :].rearrange(
                            "p h c -> p (h c)").rearrange(
                            "p (i r) -> p i r", r=256)  # [P, 8, 256]
                        for par in range(2):
                            nc.vector.scalar_tensor_tensor(
                                vblk[:, 4 * nq:4 * nq + 4,
                                     192 * par:192 * par + HD],
                                pv.rearrange("p (i r) -> p i r", r=128)[
                                    :, :, par * HD:(par + 1) * HD], ISV,
                                vbias_bc[:, nq * 512:(nq + 1) * 512].rearrange(
                                    "p (i r) -> p i r", r=128)[
                                    :, :, par * HD:(par + 1) * HD],
                                OP.mult, OP.add)

            # ------------ attention ------------
            oTp = att.enter_context(tc.tile_pool(name="oTp", bufs=1, side="right"))
            oT = oTp.tile([P, NCH, T], FP8)
            with tc.tile_pool(name="esp", bufs=2) as esp, \
                 tc.tile_pool(name="psS", bufs=2, space="PSUM") as psS, \
                 tc.tile_pool(name="psO", bufs=2, space="PSUM") as psO, \
                 tc.tile_pool(name="onp", bufs=4) as onp:
                for h in range(H):
                    hc, hf = h // 2, h % 2
                    rq = slice(64 * hf, 64 * (hf + 1))
                    es_h = esp.tile([P, NCH, T], FP8, name="es")
                    for ktc in range(NCH):
                        psc = psS.tile([P, T], F32, name="psc")
                        for qt in range(2):
                            nc.tensor.matmul(psc[:, qt * 512:(qt + 1) * 512],
                                             k_t[rq, hc, ktc * P:(ktc + 1) * P],
                                             q_t[rq, hc, qt * 512:(qt + 1) * 512],
                                             start=True, stop=True)
                        nc.scalar.activation(es_h[:, ktc, :], psc, AF.Exp,
                                             bias=eln32_c, scale=rkc[:, h, ktc:ktc + 1])
                    for qt in range(2):
                        qs = slice(qt * 512, (qt + 1) * 512)
                        po = psO.tile([P, 512], F32, name="po")
                        rs = onp.tile([P, 512], F32, name="rs")
                        rsb = onp.tile([P, 512], F32, name="rsb")
                        for kp in range(4):
                            nc.tensor.matmul(
                                po, vx[:, 2 * kp:2 * kp + 2, h, :],
                                es_h[:, 2 * kp:2 * kp + 2, qs],
                                start=(kp == 0), stop=(kp == 3), perf_mode=DR)
                        if hf == 0:
                            # denom at row 64: recip there, DMA row to
                            # partition 0, broadcast full, use rows 0:64
                            nc.vector.reciprocal(rs[64:65, :], po[64:65, :])
                            rse = onp.tile([1, 512], F32, name="rse")
                            nc.sync.dma_start(out=rse, in_=rs[64:65, :])
                            nc.gpsimd.partition_broadcast(rsb, rse)
                            nc.vector.tensor_tensor(oT[0:64, hc, qs], po[0:64, :],
                                                    rsb[0:64, :], OP.mult)
                        else:
                            # denom at row 0: broadcast full, use rows 64:128
                            nc.vector.reciprocal(rs[0:1, :], po[0:1, :])
                            nc.gpsimd.partition_broadcast(rsb, rs[0:1, :])
                            nc.vector.tensor_tensor(oT[64:128, hc, qs], po[64:128, :],
                                                    rsb[64:128, :], OP.mult)

            # ------------ proj + residual ------------
            with tc.tile_pool(name="wpp", bufs=1) as wpp, \
                 tc.tile_pool(name="psP", bufs=3, space="PSUM") as psP:
                wpj = wpp.tile([P, NCH, D], FP8, name="wpj")
                nc.sync.dma_start(out=wpj, in_=wload_ap(wproj, NCH, D, 0))
                for nt in range(2):
                    for fc in range(8):
                        pp = psP.tile([P, 512], F32, name="pp")
                        for dp in range(4):
                            nc.tensor.matmul(
                                pp, wpj[:, 2 * dp:2 * dp + 2, fc * P:(fc + 1) * P],
                                oT[:, 2 * dp:2 * dp + 2, nt * 512:(nt + 1) * 512],
                                start=(dp == 0), stop=(dp == 3), perf_mode=DR)
                        nc.vector.affine_then_add(
                            x_res[:, fc, nt * 512:(nt + 1) * 512], pp,
                            x_res[:, fc, nt * 512:(nt + 1) * 512],
                            scale=gbs_proj[:, fc:fc + 1],
                            bias=gb_proj[:, fc:fc + 1])

            att.close()  # free h1, q/k, vx, oT, rkc

            # ------------ norm2 + modulate + MLP (single pass, fp8) ------------
            with tc.tile_pool(name="h2p", bufs=1) as h2p, \
                 tc.tile_pool(name="gactp", bufs=1, side="right") as gactp:
                h2 = h2p.tile([P, NCH, T], FP8)
                norm_modulate(scl_n2, 24, 32, h2)
                gact = gactp.tile([P, MCH, T], FP8)
                with tc.tile_pool(name="wf1p", bufs=2) as wf1p, \
                     tc.tile_pool(name="psM", bufs=2, space="PSUM") as psM:
                    for mg in range(8):
                        w1 = wf1p.tile([P, NCH, 512], FP8, name="w1")
                        nc.sync.dma_start(out=w1, in_=wload_ap(wfc1, NCH, 512, mg * 512))
                        for mi in range(4):
                            m = mg * 4 + mi
                            psm = psM.tile([P, T], F32, name="psm")
                            for nt in range(2):
                                for dp in range(4):
                                    nc.tensor.matmul(
                                        psm[:, nt * 512:(nt + 1) * 512],
                                        w1[:, 2 * dp:2 * dp + 2, mi * P:(mi + 1) * P],
                                        h2[:, 2 * dp:2 * dp + 2, nt * 512:(nt + 1) * 512],
                                        start=(dp == 0), stop=(dp == 3), perf_mode=DR)
                            nc.scalar.activation(gact[:, m, :], psm,
                                                 AF.Gelu_apprx_tanh, scale=ISV,
                                                 bias=bfc1_c[:, m:m + 1])
                with tc.tile_pool(name="wf2p", bufs=2) as wf2p, \
                     tc.tile_pool(name="psM2", bufs=3, space="PSUM") as psM2:
                    for half in range(2):
                        w2 = wf2p.tile([P, MCH, 512], FP8, name="w2")
                        nc.sync.dma_start(out=w2, in_=wload_ap(wfc2, MCH, 512, half * 512))
                        for fi in range(4):
                            fc = half * 4 + fi
                            for nt in range(2):
                                ps2 = psM2.tile([P, 512], F32, name="ps2")
                                for dp in range(16):
                                    nc.tensor.matmul(
                                        ps2,
                                        w2[:, 2 * dp:2 * dp + 2, fi * P:(fi + 1) * P],
                                        gact[:, 2 * dp:2 * dp + 2,
                                             nt * 512:(nt + 1) * 512],
                                        start=(dp == 0), stop=(dp == 15), perf_mode=DR)
                                nc.vector.affine_then_add(
                                    x_res[:, fc, nt * 512:(nt + 1) * 512], ps2,
                                    x_res[:, fc, nt * 512:(nt + 1) * 512],
                                    scale=gbs_fc2[:, fc:fc + 1],
                                    bias=gb_fc2[:, fc:fc + 1])
                            nc.sync.dma_start(out=out[fc * P:(fc + 1) * P, :],
                                              in_=x_res[:, fc, :])
    nc.compile()
    return nc


_CACHE = {}


def _runner(nc, n_cores=8):
    import jax
    import numpy as _np
    from jax.sharding import Mesh, PartitionSpec, NamedSharding
    from jax.experimental.shard_map import shard_map
    from concourse.bass2jax import _bass_exec_p, install_neuronx_cc_hook, partition_id_tensor

    install_neuronx_cc_hook()
    in_names, out_names, out_avals = [], [], []
    partition_name = nc.partition_id_tensor.name if nc.partition_id_tensor else None
    for alloc in nc.m.functions[0].allocations:
        if not isinstance(alloc, mybir.MemoryLocationSet):
            continue
        nm = alloc.memorylocations[0].name
        if alloc.kind == "ExternalInput":
            if nm != partition_name:
                in_names.append(nm)
        elif alloc.kind == "ExternalOutput":
            out_names.append(nm)
            out_avals.append(jax.core.ShapedArray(tuple(alloc.tensor_shape),
                                                  mybir.dt.np(alloc.dtype)))

    def _body(*args):
        operands = list(args)
        if partition_name is not None:
            operands.append(partition_id_tensor())
        outs = _bass_exec_p.bind(
            *operands,
            out_avals=tuple(out_avals),
            in_names=tuple(in_names + [partition_name] if partition_name else in_names),
            out_names=tuple(out_names),
            lowering_input_output_aliases=(),
            sim_require_finite=False,
            sim_require_nnan=False,
            nc=nc,
        )
        return tuple(outs)

    devices = jax.devices()[:n_cores]
    mesh = Mesh(_np.asarray(devices), ("core",))
    fn = jax.jit(shard_map(_body, mesh=mesh,
                           in_specs=(PartitionSpec("core"),) * len(in_names),
                           out_specs=(PartitionSpec("core"),) * len(out_names),
                           check_rep=False))

    def run(in_maps):
        concat = [_np.concatenate([_np.asarray(m[n]) for m in in_maps], axis=0)
                  for n in in_names]
        args = [jax.device_put(c, NamedSharding(mesh, PartitionSpec("core")))
                for c in concat]
        outs = fn(*args)
        jax.block_until_ready(outs)
        res = []
        for c in range(n_cores):
            d = {}
            for i, nm in enumerate(out_names):
                full = _np.asarray(outs[i])
                d[nm] = full.reshape(n_cores, *out_avals[i].shape)[c]
            res.append(d)
        return res

    return run


def kernel(**inputs):
    """Full (unsharded) inputs -> full (B, T, D) float32 output."""
    if "nc" not in _CACHE:
        _CACHE["nc"] = build_dit(n_cores=8)
        _CACHE["run"] = _runner(_CACHE["nc"], 8)
    in_maps = host_prep(**inputs)
    results = _CACHE["run"](in_maps)
    return host_post(results)


# revision 32
# speedup vs baseline: 2.1580x; 1.0262x over previous
"""nn_DiTBlock on 8 TRN2 NeuronCores: data-parallel over batch (B=8), one
batch element per core. Self-contained: builds the Bass/Tile kernel, shards
inputs on the host (transpose/pack/cast only), runs SPMD via bass2jax/PJRT,
gathers and un-transposes the output.

v2 design: fp8e4(e4m3)+DoubleRow matmuls for qkv/v/attn@v/proj/fc1/fc2
(weights host-prescaled x128, descale folded into evictions), bf16 for adaLN
and attention scores, f32 residual + PSUM. exp scaled by 1/32 (cancels in
softmax); k-rmsnorm folded into the exp's per-partition scale; odd heads'
attn@v written directly to PSUM partitions 64:128 (no partition-move DMAs);
single-pass MLP; multi-chunk batched weight DMAs."""

import numpy as np
from contextlib import ExitStack

import concourse.bass as bass
import concourse.mybir as mybir
import concourse.tile as tile
from concourse import bacc


F32 = mybir.dt.float32
F32R = mybir.dt.float32r
BF16 = mybir.dt.bfloat16
FP8 = mybir.dt.float8e4
AF = mybir.ActivationFunctionType
OP = mybir.AluOpType
DR = mybir.MatmulPerfMode.DoubleRow

B, T, D, H = 8, 1024, 1024, 16
HD = D // H          # 64
DM = 4 * D           # 4096
NCH = D // 128       # 8
MCH = DM // 128      # 32
P = 128
WS = 128.0           # fp8 weight pre-scale (host)
ISV = 1.0 / WS
ELN32 = -3.4657359027997265  # -ln(32): exp pre-scale so fp8 es stays < 240


def host_prep(x, c, g1, g2, gq, gk, Wqkv, bqkv, Wproj, bproj,
              Wfc1, bfc1, Wfc2, bfc2, Wada, bada):
    import ml_dtypes
    E4 = mybir.dt.np(FP8)

    def packT(W, npdt, scale=1.0):  # (F, K) -> (K//128, 128, F) contiguous
        Wt = np.ascontiguousarray(np.asarray(W, np.float32).T * scale).astype(npdt)
        K, F = Wt.shape
        return np.ascontiguousarray(Wt.reshape(K // 128, 128, F))

    f32 = np.float32
    com = {
        "wqkv": packT(Wqkv, E4, WS), "wproj": packT(Wproj, E4, WS),
        "wfc1": packT(Wfc1, E4, WS), "wfc2": packT(Wfc2, E4, WS),
        "wada": packT(Wada, ml_dtypes.bfloat16),
        "bqkv": np.asarray(bqkv, f32), "bproj": np.asarray(bproj, f32),
        "bfc1": np.asarray(bfc1, f32), "bfc2": np.asarray(bfc2, f32),
        "bada": np.asarray(bada, f32),
        "g": np.stack([np.asarray(g1)[0], np.asarray(g2)[0],
                       np.asarray(gq)[0], np.asarray(gk)[0]]).astype(f32),
    }
    in_maps = []
    for b in range(B):
        m = dict(com)
        m["xt"] = np.ascontiguousarray(np.asarray(x[b], f32).T)
        m["cvec"] = np.asarray(c[b], f32)
        in_maps.append(m)
    return in_maps


def host_post(results):
    return np.ascontiguousarray(
        np.stack([r["out"].T for r in results]).astype(np.float32))


def col_ap(handle, nch):
    """DRAM (nch*128,) viewed as [128, nch]: tile[p, ch] = v[ch*128+p]."""
    return bass.AP(tensor=handle, offset=0, ap=[[1, P], [P, nch]])


def bc_ap(handle, n, offset=0):
    """DRAM (n,) broadcast-read to [128, n] (partition stride 0)."""
    return bass.AP(tensor=handle, offset=offset, ap=[[0, P], [1, n]])


def wload_ap(handle, kch, cols, col0):
    """DRAM weight pack [KCH,128,F] -> [128, kch, cols] AP at col offset."""
    F = handle.shape[2]
    return bass.AP(tensor=handle, offset=col0,
                   ap=[[F, P], [P * F, kch], [1, cols]])


def build_dit(n_cores=8):
    nc = bacc.Bacc("TRN2", target_bir_lowering=False, debug=False,
                   num_devices=n_cores)

    xt = nc.dram_tensor("xt", [D, T], F32, kind="ExternalInput")
    cin = nc.dram_tensor("cvec", [D], F32, kind="ExternalInput")
    g = nc.dram_tensor("g", [4], F32, kind="ExternalInput")
    wqkv = nc.dram_tensor("wqkv", [NCH, P, 3 * D], FP8, kind="ExternalInput")
    wproj = nc.dram_tensor("wproj", [NCH, P, D], FP8, kind="ExternalInput")
    wfc1 = nc.dram_tensor("wfc1", [NCH, P, DM], FP8, kind="ExternalInput")
    wfc2 = nc.dram_tensor("wfc2", [MCH, P, D], FP8, kind="ExternalInput")
    wada = nc.dram_tensor("wada", [NCH, P, 6 * D], BF16, kind="ExternalInput")
    bqkv = nc.dram_tensor("bqkv", [3 * D], F32, kind="ExternalInput")
    bproj = nc.dram_tensor("bproj", [D], F32, kind="ExternalInput")
    bfc1 = nc.dram_tensor("bfc1", [DM], F32, kind="ExternalInput")
    bfc2 = nc.dram_tensor("bfc2", [D], F32, kind="ExternalInput")
    bada = nc.dram_tensor("bada", [6 * D], F32, kind="ExternalInput")
    out = nc.dram_tensor("out", [D, T], F32, kind="ExternalOutput")

    with tile.TileContext(nc, pool_alloc_mode="queue") as tc:
        with ExitStack() as X:
            const = X.enter_context(tc.tile_pool(name="const", bufs=1))
            resid = X.enter_context(tc.tile_pool(name="resid", bufs=1))
            dram = X.enter_context(tc.tile_pool(name="dram", bufs=1, space="DRAM"))

            # ---------------- constants ----------------
            g_bc = const.tile([P, 4], F32)
            nc.sync.dma_start(out=g_bc, in_=bc_ap(g, 4))
            gsq = const.tile([P, 4], F32)
            nc.vector.tensor_tensor(gsq, g_bc, g_bc, OP.mult)
            ginv2 = const.tile([P, 4], F32)
            nc.vector.reciprocal(ginv2, gsq)
            # Rsqrt scales: rinv = rsqrt(ps * scl)
            scl_n1 = const.tile([P, 1], F32)
            nc.vector.tensor_scalar_mul(scl_n1, ginv2[:, 0:1], 1.0 / D)
            scl_n2 = const.tile([P, 1], F32)
            nc.vector.tensor_scalar_mul(scl_n2, ginv2[:, 1:2], 1.0 / D)
            scl_q = const.tile([P, 1], F32)
            nc.vector.tensor_copy(scl_q, ginv2[:, 2:3])
            scl_k = const.tile([P, 1], F32)
            nc.vector.tensor_scalar_mul(scl_k, ginv2[:, 3:4], 1.0 / HD)

            ones1_f = const.tile([P, 1], F32)
            nc.gpsimd.memset(ones1_f, 1.0)
            ones1 = const.tile([P, 1], BF16)
            nc.vector.tensor_copy(ones1, ones1_f)
            onesh_f = const.tile([P, 2], F32)
            nc.gpsimd.memset(onesh_f, 0.0)
            nc.gpsimd.memset(onesh_f[0:64, 0:1], 1.0)
            nc.gpsimd.memset(onesh_f[64:128, 1:2], 1.0)
            onesh = const.tile([P, 2], BF16)
            nc.vector.tensor_copy(onesh, onesh_f)

            bqkv_c = const.tile([P, 3 * D // P], F32)
            nc.sync.dma_start(out=bqkv_c, in_=col_ap(bqkv, 3 * D // P))
            bproj_c = const.tile([P, NCH], F32)
            nc.sync.dma_start(out=bproj_c, in_=col_ap(bproj, NCH))
            bfc1_c = const.tile([P, MCH], F32)
            nc.sync.dma_start(out=bfc1_c, in_=col_ap(bfc1, MCH))
            bfc2_c = const.tile([P, NCH], F32)
            nc.sync.dma_start(out=bfc2_c, in_=col_ap(bfc2, NCH))
            vbias_bc = const.tile([P, D], F32)
            nc.sync.dma_start(out=vbias_bc, in_=bc_ap(bqkv, D, offset=2 * D))
            eln32_c = const.tile([P, 1], F32)
            nc.gpsimd.memset(eln32_c, ELN32)

            x_res = resid.tile([P, NCH, T], F32)
            for j in range(4):
                nc.sync.dma_start(
                    out=x_res[:, 2 * j:2 * j + 2, :],
                    in_=bass.AP(tensor=xt, offset=2 * j * P * T,
                                ap=[[T, P], [P * T, 2], [1, T]]))

            c_pm = const.tile([P, NCH], F32)
            nc.sync.dma_start(out=c_pm, in_=col_ap(cin, NCH))
            cs_pm = const.tile([P, NCH], BF16)
            nc.scalar.activation(cs_pm, c_pm, AF.Silu)

            # ---------------- adaLN (bf16) ----------------
            ada_scr = dram.tile([1, 6 * D], F32)
            ada_sb = const.tile([1, 6 * D], F32)
            with tc.tile_pool(name="wadap", bufs=3) as wp, \
                 tc.tile_pool(name="psA", bufs=2, space="PSUM") as psA:
                for nb in range(12):
                    wt = wp.tile([P, NCH, 512], BF16, name="wt")
                    nc.sync.dma_start(out=wt, in_=wload_ap(wada, NCH, 512, nb * 512))
                    pa = psA.tile([1, 512], F32, name="pa")
                    for d in range(NCH):
                        nc.tensor.matmul(pa, cs_pm[:, d:d + 1], wt[:, d, :],
                                         start=(d == 0), stop=(d == NCH - 1))
                    nc.vector.tensor_copy(ada_sb[:, nb * 512:(nb + 1) * 512], pa)
            nc.sync.dma_start(out=ada_scr, in_=ada_sb)
            adaT = const.tile([P, 48], F32)
            nc.sync.dma_start(out=adaT, in_=bass.AP(tensor=ada_scr.tensor, offset=0,
                                                    ap=[[1, P], [P, 48]]))
            badaT = const.tile([P, 48], F32)
            nc.sync.dma_start(out=badaT, in_=col_ap(bada, 48))
            nc.vector.tensor_tensor(adaT, adaT, badaT, OP.add)
            # cols: shift_msa 0:8 | scale_msa 8:16 | gate_msa 16:24
            #       shift_mlp 24:32 | scale_mlp 32:40 | gate_mlp 40:48
            nc.vector.tensor_scalar_add(adaT[:, 8:16], adaT[:, 8:16], 1.0)
            nc.vector.tensor_scalar_add(adaT[:, 32:40], adaT[:, 32:40], 1.0)
            gb_proj = const.tile([P, NCH], F32)
            nc.vector.tensor_tensor(gb_proj, adaT[:, 16:24], bproj_c, OP.mult)
            gbs_proj = const.tile([P, NCH], F32)
            nc.vector.tensor_scalar_mul(gbs_proj, adaT[:, 16:24], ISV)
            gb_fc2 = const.tile([P, NCH], F32)
            nc.vector.tensor_tensor(gb_fc2, adaT[:, 40:48], bfc2_c, OP.mult)
            gbs_fc2 = const.tile([P, NCH], F32)
            nc.vector.tensor_scalar_mul(gbs_fc2, adaT[:, 40:48], ISV)

            def norm_modulate(scl, sh_col, sc_col, h_out):
                """x_res (f32) -> h_out (fp8): rmsnorm + adaLN modulate.
                Token-halved so the consumer can start on half 0 while the
                producer of x_res is still finishing half 1."""
                with tc.tile_pool(name="sqp", bufs=3) as sqp, \
                     tc.tile_pool(name="psN", bufs=1, space="PSUM") as psN, \
                     tc.tile_pool(name="nrm", bufs=2) as nrm, \
                     tc.tile_pool(name="xnp", bufs=3) as xnp:
                    pss = psN.tile([1, T], F32, name="pss")
                    for t2 in range(2):
                        ts_ = slice(t2 * 512, (t2 + 1) * 512)
                        for j in range(NCH):
                            xsq = sqp.tile([P, 512], BF16, name="xsq")
                            nc.scalar.activation(xsq, x_res[:, j, ts_], AF.Square)
                            nc.tensor.matmul(pss[:, ts_], ones1, xsq,
                                             start=(j == 0), stop=(j == NCH - 1))
                        rr = nrm.tile([1, 512], F32, name="rr")
                        nc.scalar.activation(rr, pss[:, ts_], AF.Sqrt,
                                             scale=scl[0:1, :])
                        rinv = nrm.tile([1, 512], F32, name="rinv")
                        nc.vector.reciprocal(rinv, rr)
                        rbc = nrm.tile([P, 512], F32, name="rbc")
                        nc.gpsimd.partition_broadcast(rbc, rinv)
                        for j in range(NCH):
                            xn = xnp.tile([P, 512], F32, name="xn")
                            nc.vector.tensor_tensor(xn, x_res[:, j, ts_], rbc,
                                                    OP.mult)
                            nc.gpsimd.tensor_scalar(h_out[:, j, ts_], xn,
                                                    adaT[:, sc_col + j:sc_col + j + 1],
                                                    adaT[:, sh_col + j:sh_col + j + 1],
                                                    OP.mult, OP.add)

            # fc1 weights tile created before the attention pools (so they
            # can close first); its load is emitted at proj time, landing
            # during attention when the wire is idle
            mlpw = X.enter_context(tc.tile_pool(name="mlpw", bufs=1))
            w1a = mlpw.tile([P, NCH, DM // 2], FP8)

            att = ExitStack()
            h1p = att.enter_context(tc.tile_pool(name="h1p", bufs=1, side="right"))
            h1 = h1p.tile([P, NCH, T], FP8)
            # ------------ norm1 + modulate ------------
            norm_modulate(scl_n1, 0, 8, h1)

            # ------------ q, k (feature-major bf16) + per-head rmsnorm ------------
            qp_ = att.enter_context(tc.tile_pool(name="qp_", bufs=1))
            kp_ = att.enter_context(tc.tile_pool(name="kp_", bufs=1))
            q_t = qp_.tile([P, NCH, T], BF16)
            k_t = kp_.tile([P, NCH, T], BF16)
            rkcp = att.enter_context(tc.tile_pool(name="rkcp", bufs=1))
            rkc = rkcp.tile([P, H, NCH], F32)  # 1/|k| per k-token, head-major

            with tc.tile_pool(name="wqp", bufs=2) as wqp, \
                 tc.tile_pool(name="sqq", bufs=2) as sqq, \
                 tc.tile_pool(name="psD", bufs=2, space="PSUM") as psD, \
                 tc.tile_pool(name="psR", bufs=1, space="PSUM") as psR, \
                 tc.tile_pool(name="nrq", bufs=2) as nrq:
                for fc in range(16):  # q: 0..7, k: 8..15
                    if fc % 4 == 0:
                        wt = wqp.tile([P, NCH, 512], FP8, name="wt")
                        nc.sync.dma_start(out=wt, in_=wload_ap(wqkv, NCH, 512, fc * P))
                    tgt = q_t if fc < 8 else k_t
                    ch = fc % 8
                    ps = [psD.tile([P, 512], F32, name="ps") for _ in range(2)]
                    for nt in range(2):
                        for dp in range(4):
                            nc.tensor.matmul(
                                ps[nt],
                                wt[:, 2 * dp:2 * dp + 2, (fc % 4) * P:(fc % 4 + 1) * P],
                                h1[:, 2 * dp:2 * dp + 2, nt * 512:(nt + 1) * 512],
                                start=(dp == 0), stop=(dp == 3), perf_mode=DR)
                        # evict: (psum/128 + bias) -> bf16 on Act (idle here)
                        nc.scalar.activation(tgt[:, ch, nt * 512:(nt + 1) * 512],
                                             ps[nt], AF.Identity, scale=ISV,
                                             bias=bqkv_c[:, fc:fc + 1])
                    # sum of squares per head
                    sq = sqq.tile([P, T], BF16, name="sq")
                    nc.vector.tensor_tensor(sq, tgt[:, ch, :], tgt[:, ch, :], OP.mult)
                    if fc < 8:
                        # q: per-half sums, each in its own row-0 psum tile;
                        # broadcasts always source partition 0 into full tiles
                        for hfq in range(2):
                            prh = psR.tile([1, T], F32, name=f"prh{hfq}")
                            for nt in range(2):
                                nc.tensor.matmul(
                                    prh[:, nt * 512:(nt + 1) * 512],
                                    onesh[:, hfq:hfq + 1],
                                    sq[:, nt * 512:(nt + 1) * 512],
                                    start=True, stop=True)
                            rr2 = nrq.tile([1, T], BF16, name=f"rr2{hfq}")
                            nc.scalar.activation(rr2, prh, AF.Sqrt,
                                                 scale=scl_q[0:1, :])
                            rinv_sb = nrq.tile([1, T], BF16, name=f"ri{hfq}")
                            with nc.allow_low_precision(reason="1/|q| bf16"):
                                nc.vector.reciprocal(rinv_sb, rr2)
                            rbcq = nrq.tile([P, T], BF16, name=f"rbcq{hfq}")
                            nc.gpsimd.partition_broadcast(rbcq, rinv_sb)
                            hs = slice(64 * hfq, 64 * (hfq + 1))
                            nc.vector.tensor_tensor(q_t[hs, ch, :], q_t[hs, ch, :],
                                                    rbcq[hs, :], OP.mult)
                    else:
                        # k: sums token-major [128, NCH] per head -> 1/|k| into
                        # rkc, consumed as the exp's per-partition scale.
                        for j in range(2):
                            hidx = 2 * (fc - 8) + j
                            pkn = psR.tile([P, NCH], F32, name="pkn")
                            for kt in range(NCH):
                                nc.tensor.matmul(
                                    pkn[:, kt:kt + 1],
                                    sq[64 * j:64 * (j + 1), kt * P:(kt + 1) * P],
                                    ones1[64 * j:64 * (j + 1), :],
                                    start=True, stop=True)
                            rrk = nrq.tile([P, NCH], F32, name="rrk")
                            nc.scalar.activation(rrk, pkn, AF.Sqrt, scale=scl_k)
                            nc.vector.reciprocal(rkc[:, hidx, :], rrk)

            # ------------ v (token-major fp8, ones-augmented) ------------
            # vx per-head 128-col slot: even h = [v(0:64) | ones@64 | 0],
            # odd h = [0 | ones@63 | v(64:128)]; attn@v DR outputs are then
            # always full [128, N] (walrus requires that) and odd heads land
            # on PSUM partitions 64:128 directly.
            vxp = att.enter_context(tc.tile_pool(name="vxp", bufs=1))
            vx = vxp.tile([P, NCH, H, P], FP8)   # [ktok][ktc][head][col]
            nc.gpsimd.memset(vx, 0.0)
            for h in range(H):
                oc = HD if h % 2 == 0 else 0
                nc.gpsimd.memset(vx[:, :, h, oc:oc + 1], 1.0)
            with tc.tile_pool(name="wvp", bufs=2) as wvp, \
                 tc.tile_pool(name="psV", bufs=3, space="PSUM") as psV:
                for nq in range(2):
                    wv = wvp.tile([P, NCH, 512], FP8, name="wv")
                    nc.sync.dma_start(out=wv,
                                      in_=wload_ap(wqkv, NCH, 512, 2 * D + nq * 512))
                    for t8 in range(NCH):
                        pv = psV.tile([P, 512], F32, name="pv")
                        for dp in range(4):
                            nc.tensor.matmul(
                                pv, h1[:, 2 * dp:2 * dp + 2, t8 * P:(t8 + 1) * P],
                                wv[:, 2 * dp:2 * dp + 2, :],
                                start=(dp == 0), stop=(dp == 3), perf_mode=DR)
                        # heads alternate col-base 0 (even) / 64 (odd) in vx
                        vblk = vx[:, t8, :, :].rearrange(
                            "p h c -> p (h c)").rearrange(
                            "p (i r) -> p i r", r=256)  # [P, 8, 256]
                        for par in range(2):
                            nc.vector.scalar_tensor_tensor(
                                vblk[:, 4 * nq:4 * nq + 4,
                                     192 * par:192 * par + HD],
                                pv.rearrange("p (i r) -> p i r", r=128)[
                                    :, :, par * HD:(par + 1) * HD], ISV,
                                vbias_bc[:, nq * 512:(nq + 1) * 512].rearrange(
                                    "p (i r) -> p i r", r=128)[
                                    :, :, par * HD:(par + 1) * HD],
                                OP.mult, OP.add)

            # ------------ attention ------------
            oTp = att.enter_context(tc.tile_pool(name="oTp", bufs=1, side="right"))
            oT = oTp.tile([P, NCH, T], FP8)
            with tc.tile_pool(name="esp", bufs=2) as esp, \
                 tc.tile_pool(name="psS", bufs=2, space="PSUM") as psS, \
                 tc.tile_pool(name="psO", bufs=2, space="PSUM") as psO, \
                 tc.tile_pool(name="onp", bufs=4) as onp:
                for h in range(H):
                    hc, hf = h // 2, h % 2
                    rq = slice(64 * hf, 64 * (hf + 1))
                    es_h = esp.tile([P, NCH, T], FP8, name="es")
                    for ktc in range(NCH):
                        psc = psS.tile([P, T], F32, name="psc")
                        for qt in range(2):
                            nc.tensor.matmul(psc[:, qt * 512:(qt + 1) * 512],
                                             k_t[rq, hc, ktc * P:(ktc + 1) * P],
                                             q_t[rq, hc, qt * 512:(qt + 1) * 512],
                                             start=True, stop=True)
                        nc.scalar.activation(es_h[:, ktc, :], psc, AF.Exp,
                                             bias=eln32_c, scale=rkc[:, h, ktc:ktc + 1])
                    for qt in range(2):
                        qs = slice(qt * 512, (qt + 1) * 512)
                        po = psO.tile([P, 512], F32, name="po")
                        rs = onp.tile([P, 512], F32, name="rs")
                        rsb = onp.tile([P, 512], F32, name="rsb")
                        for kp in range(4):
                            nc.tensor.matmul(
                                po, vx[:, 2 * kp:2 * kp + 2, h, :],
                                es_h[:, 2 * kp:2 * kp + 2, qs],
                                start=(kp == 0), stop=(kp == 3), perf_mode=DR)
                        if hf == 0:
                            # denom at row 64: recip there, DMA row to
                            # partition 0, broadcast full, use rows 0:64
                            nc.vector.reciprocal(rs[64:65, :], po[64:65, :])
                            rse = onp.tile([1, 512], F32, name="rse")
                            nc.sync.dma_start(out=rse, in_=rs[64:65, :])
                            nc.gpsimd.partition_broadcast(rsb, rse)
                            nc.vector.tensor_tensor(oT[0:64, hc, qs], po[0:64, :],
                                                    rsb[0:64, :], OP.mult)
                        else:
                            # denom at row 0: broadcast full, use rows 64:128
                            nc.vector.reciprocal(rs[0:1, :], po[0:1, :])
                            nc.gpsimd.partition_broadcast(rsb, rs[0:1, :])
                            nc.vector.tensor_tensor(oT[64:128, hc, qs], po[64:128, :],
                                                    rsb[64:128, :], OP.mult)

            # ------------ proj + residual ------------
            with tc.tile_pool(name="wpp", bufs=1) as wpp, \
                 tc.tile_pool(name="psP", bufs=3, space="PSUM") as psP:
                wpj = wpp.tile([P, NCH, D], FP8, name="wpj")
                nc.sync.dma_start(out=wpj, in_=wload_ap(wproj, NCH, D, 0))
                nc.sync.dma_start(out=w1a, in_=wload_ap(wfc1, NCH, DM // 2, 0))
                for nt in range(2):
                    for fc in range(8):
                        pp = psP.tile([P, 512], F32, name="pp")
                        for dp in range(4):
                            nc.tensor.matmul(
                                pp, wpj[:, 2 * dp:2 * dp + 2, fc * P:(fc + 1) * P],
                                oT[:, 2 * dp:2 * dp + 2, nt * 512:(nt + 1) * 512],
                                start=(dp == 0), stop=(dp == 3), perf_mode=DR)
                        nc.vector.affine_then_add(
                            x_res[:, fc, nt * 512:(nt + 1) * 512], pp,
                            x_res[:, fc, nt * 512:(nt + 1) * 512],
                            scale=gbs_proj[:, fc:fc + 1],
                            bias=gb_proj[:, fc:fc + 1])

            att.close()  # free h1, q/k, vx, oT, rkc

            # ------------ norm2 + modulate + MLP (single pass, fp8) ------------
            with tc.tile_pool(name="h2p", bufs=1) as h2p, \
                 tc.tile_pool(name="gactp", bufs=1, side="right") as gactp:
                h2 = h2p.tile([P, NCH, T], FP8)
                norm_modulate(scl_n2, 24, 32, h2)
                gact = gactp.tile([P, MCH, T], FP8)
                w1b = gactp.tile([P, NCH, DM // 2], FP8, name="w1b")
                nc.sync.dma_start(out=w1b, in_=wload_ap(wfc1, NCH, DM // 2, DM // 2))
                w2 = gactp.tile([P, MCH, D], FP8, name="w2full")
                nc.sync.dma_start(out=w2, in_=wload_ap(wfc2, MCH, D, 0))
                # nt-outer: fc2 on token-half 0 overlaps fc1/gelu on half 1
                with tc.tile_pool(name="psM", bufs=3, space="PSUM") as psM, \
                     tc.tile_pool(name="psM2", bufs=3, space="PSUM") as psM2:
                    for nt in range(2):
                        ns_ = slice(nt * 512, (nt + 1) * 512)
                        for m in range(MCH):
                            psm = psM.tile([P, 512], F32, name="psm")
                            w1h = w1a if m < 16 else w1b
                            mo = m if m < 16 else m - 16
                            for dp in range(4):
                                nc.tensor.matmul(
                                    psm,
                                    w1h[:, 2 * dp:2 * dp + 2, mo * P:(mo + 1) * P],
                                    h2[:, 2 * dp:2 * dp + 2, ns_],
                                    start=(dp == 0), stop=(dp == 3), perf_mode=DR)
                            nc.scalar.activation(gact[:, m, ns_], psm,
                                                 AF.Gelu_apprx_tanh, scale=ISV,
                                                 bias=bfc1_c[:, m:m + 1])
                        for fc in range(8):
                            ps2 = psM2.tile([P, 512], F32, name="ps2")
                            for dp in range(16):
                                nc.tensor.matmul(
                                    ps2,
                                    w2[:, 2 * dp:2 * dp + 2, fc * P:(fc + 1) * P],
                                    gact[:, 2 * dp:2 * dp + 2, ns_],
                                    start=(dp == 0), stop=(dp == 15), perf_mode=DR)
                            nc.vector.affine_then_add(
                                x_res[:, fc, ns_], ps2, x_res[:, fc, ns_],
                                scale=gbs_fc2[:, fc:fc + 1],
                                bias=gb_fc2[:, fc:fc + 1])
                            if nt == 1:
                                nc.sync.dma_start(out=out[fc * P:(fc + 1) * P, :],
                                                  in_=x_res[:, fc, :])
    nc.compile()
    return nc


_CACHE = {}


def _runner(nc, n_cores=8):
    import jax
    import numpy as _np
    from jax.sharding import Mesh, PartitionSpec, NamedSharding
    from jax.experimental.shard_map import shard_map
    from concourse.bass2jax import _bass_exec_p, install_neuronx_cc_hook, partition_id_tensor

    install_neuronx_cc_hook()
    in_names, out_names, out_avals = [], [], []
    partition_name = nc.partition_id_tensor.name if nc.partition_id_tensor else None
    for alloc in nc.m.functions[0].allocations:
        if not isinstance(alloc, mybir.MemoryLocationSet):
            continue
        nm = alloc.memorylocations[0].name
        if alloc.kind == "ExternalInput":
            if nm != partition_name:
                in_names.append(nm)
        elif alloc.kind == "ExternalOutput":
            out_names.append(nm)
            out_avals.append(jax.core.ShapedArray(tuple(alloc.tensor_shape),
                                                  mybir.dt.np(alloc.dtype)))

    def _body(*args):
        operands = list(args)
        if partition_name is not None:
            operands.append(partition_id_tensor())
        outs = _bass_exec_p.bind(
            *operands,
            out_avals=tuple(out_avals),
            in_names=tuple(in_names + [partition_name] if partition_name else in_names),
            out_names=tuple(out_names),
            lowering_input_output_aliases=(),
            sim_require_finite=False,
            sim_require_nnan=False,
            nc=nc,
        )
        return tuple(outs)

    devices = jax.devices()[:n_cores]
    mesh = Mesh(_np.asarray(devices), ("core",))
    fn = jax.jit(shard_map(_body, mesh=mesh,
                           in_specs=(PartitionSpec("core"),) * len(in_names),
                           out_specs=(PartitionSpec("core"),) * len(out_names),
                           check_rep=False))

    def run(in_maps):
        concat = [_np.concatenate([_np.asarray(m[n]) for m in in_maps], axis=0)
                  for n in in_names]
        args = [jax.device_put(c, NamedSharding(mesh, PartitionSpec("core")))
                for c in concat]
        outs = fn(*args)
        jax.block_until_ready(outs)
        res = []
        for c in range(n_cores):
            d = {}
            for i, nm in enumerate(out_names):
                full = _np.asarray(outs[i])
                d[nm] = full.reshape(n_cores, *out_avals[i].shape)[c]
            res.append(d)
        return res

    return run


def kernel(**inputs):
    """Full (unsharded) inputs -> full (B, T, D) float32 output."""
    if "nc" not in _CACHE:
        _CACHE["nc"] = build_dit(n_cores=8)
        _CACHE["run"] = _runner(_CACHE["nc"], 8)
    in_maps = host_prep(**inputs)
    results = _CACHE["run"](in_maps)
    return host_post(results)
